# revision 1
# baseline (speedup 1.0000x reference)
"""Differential attention (B=2, T=2048, C=2048, 8 heads x 256) on 8 trn2 cores.

Sharding: tensor-parallel over the 8 effective heads — core h computes head h's
projections + attention and a partial output projection; host sums partials.

Per-core layouts (bf16 matmuls, f32 PSUM):
  xT      [C, B*T]      (host-transposed input, shared)
  wqkvT   [C, 768]      (head slice of wq|wk|wv, host-transposed)
  woT     [256, C]      (head slice of wo * (1-lambda_init), host-transposed)
  Q,K,V come out [tok, d]; Q/K rms-normed then transposed to [d, tok].
  Scores computed transposed: S.T[kk, q] = K_tile.T @ Q  -> exp -> P.T tiles
  feed PV matmul as lhsT directly (no P transpose). Ones-column on V gives the
  softmax denominator from the same matmul. Causal blocks skipped; diagonal
  blocks masked multiplicatively post-exp (scores bounded by +-sqrt(128), so
  softmax needs no max subtraction).
"""

import math
from contextlib import ExitStack

import numpy as np

# ---- problem constants (hardcoded per the harness contract) ----
B = 2
T = 2048
C = 2048
N_HEAD = 8
HEAD_DIM = 256
HALF = 128
LAMBDA_INIT = 0.8
RMS_EPS = 1.1920929e-07
N_CORES = 8

P = 128          # partitions
TOK_CHUNK = 512  # projection tok chunk (DMA granularity)

DEFAULT_OPTS = dict(
    att_chunk=256,       # attention q-chunk width (256 or 512)
    qk_tr="pe",          # "pe" | "dma": Q/K transpose path
    y_tr="pe",           # "pe" | "dma": y transpose path
    oproj_copy="alt",    # out-proj PSUM->SBUF evacuation: "act"|"dve"|"alt"
    psum=(3, 3, 2),      # banks: (proj, st, y) — must sum to <= 8
    tr_dma_engine="scalar",  # HWDGE queue for DMA transposes
    pt_bufs=5,           # P.T tile double-buffer depth
    xc_bufs=2,           # x chunk prefetch depth
    vcopy="dve",         # "act" | "dve": V PSUM->SBUF copy engine
    split_dma=False,     # split weight/first-chunk DMAs per c-tile (fast ramp)
    xc_first=True,       # issue the first x chunk's DMA before the weights
    osb_merge=True,      # one output-store DMA per tok block (vs per c-chunk)
    rms_batch=1,         # 1 | 2: tok-blocks sharing one Sqrt (fewer ACT table switches)
    rms_mode="newton",   # "sqrt" (ACT Sqrt) | "newton" (DVE-only rsqrt)
    narrow_top=True,     # compute only the valid half of the top causal row
    out_dma_alt=False,   # alternate output stores between sync/scalar queues
    tr_pool="st",        # "st" | "pp": PSUM pool used by PE transposes
    k_major=True,        # produce K in [d, tok] layout directly; rms-norm of K
                         # folded into the exp scale (per-partition AP)
    ramp_mini=False,     # dedicate a 128-tok mini DMA + q|v weight half to the
                         # very first block so PE starts ~2x earlier
    tail_split=False,    # stream the final block's stores per c-chunk
    rms_bufs=4,
    qn_bufs=6,
    y0_mult=2,
    ksq_eng="dve",       # "act" | "dve": engine computing k^2 (k_major)
    ktcopy_eng="dve",    # "act" | "dve": engine evacuating KT psum (k_major)
    ytr_pool="y",        # "st" | "y": PSUM pool for the y transposes
    ksq_src="sbuf",      # "psum" | "sbuf": k^2 input (sbuf frees KT psum sooner)
    chunk_order="asc",   # "asc" | "desc": attention q-chunk processing order
    wsplit=False,        # split wqkv DMA: q|v half first, k third deferred
)


def build_nc(c_dim, t_dim, b_dim, **opts):
    """Build the per-core Bass module. All shapes in tokens/channels."""
    import concourse.mybir as mybir
    import concourse.tile as tile
    from concourse import bacc
    from concourse.masks import make_identity, make_upper_triangular

    o = dict(DEFAULT_OPTS)
    o.update(opts)
    QCH = o["att_chunk"]
    jpc = QCH // P  # j-blocks per attention chunk

    dt = mybir.dt
    f32 = dt.float32
    bf16 = dt.bfloat16
    AF = mybir.ActivationFunctionType
    OP = mybir.AluOpType

    n_ctiles = c_dim // P            # contraction tiles over C
    ntok = b_dim * t_dim             # total token rows
    n_blocks_b = t_dim // P          # 128-tok blocks per batch
    n_qchunks = t_dim // QCH         # attention q chunks per batch
    blocks_per_chunk = TOK_CHUNK // P
    inv_sqrt_half = 1.0 / math.sqrt(HALF)
    VP = 272                         # V tile pitch (256 vals + 1 ones + pad)

    nc = bacc.Bacc()
    xt = nc.declare_dram_parameter("xt", [c_dim, ntok], bf16, isOutput=False)
    wqkv = nc.declare_dram_parameter("wqkv", [c_dim, 3 * HEAD_DIM], bf16, isOutput=False)
    wot = nc.declare_dram_parameter("wot", [HEAD_DIM, c_dim], bf16, isOutput=False)
    lamneg = nc.declare_dram_parameter("lamneg", [P, 1], f32, isOutput=False)
    out = nc.declare_dram_parameter("out", [ntok, c_dim], f32, isOutput=True)

    xt_r = xt.ap().rearrange("(i p) t -> p i t", p=P)      # [128, n_ctiles, ntok]
    wqkv_r = wqkv.ap().rearrange("(i p) n -> p i n", p=P)  # [128, n_ctiles, 768]
    wot_r = wot.ap().rearrange("(e p) n -> p e n", p=P)    # [128, 2, c_dim]

    tr_eng = nc.scalar if o["tr_dma_engine"] == "scalar" else nc.sync

    with tile.TileContext(nc) as tc:
        with ExitStack() as ctx:
            # ---- persistent SBUF ----
            const_pool = ctx.enter_context(tc.tile_pool(name="const", bufs=1))
            wqkv_sb = const_pool.tile([P, n_ctiles, 3 * HEAD_DIM], bf16, name="wqkv_sb")
            wot_sb = const_pool.tile([P, 2, c_dim], bf16, name="wot_sb")
            lam_sb = const_pool.tile([P, 1], f32, name="lam_sb")
            ident = const_pool.tile([P, P], bf16, name="ident")
            trimask = const_pool.tile([P, P], bf16, name="trimask")
            ones_sb = const_pool.tile([P, 1], bf16, name="ones_sb")
            nc.vector.memset(ones_sb[:], 1.0)

            # ---- pools ----
            xc_pool = ctx.enter_context(tc.tile_pool(name="xc", bufs=o["xc_bufs"]))

            xc0 = None
            xcmini = None
            if o["ramp_mini"] and o["k_major"]:
                # smallest possible ramp: 0.5 MB of x + the q|v weight half,
                # so block 0's matmuls can start and finish early
                xcmini = const_pool.tile([P, n_ctiles, P], bf16, name="xcmini")
                nc.sync.dma_start(xcmini[:], xt_r[:, :, 0:P])
                nc.sync.dma_start(wqkv_sb[:, :, 0:512], wqkv_r[:, :, 0:512])
                xc0 = xc_pool.tile([P, n_ctiles, TOK_CHUNK], bf16, tag="xc",
                                   name="xc0")
                nc.sync.dma_start(xc0[:], xt_r[:, :, 0:TOK_CHUNK])
                nc.sync.dma_start(wqkv_sb[:, :, 512:768], wqkv_r[:, :, 512:768])
                nc.sync.dma_start(wot_sb[:], wot_r[:])
            else:
                if o["xc_first"]:
                    # the very first DMA in the queue is the data the first
                    # matmul needs; weights follow immediately after
                    xc0 = xc_pool.tile([P, n_ctiles, TOK_CHUNK], bf16, tag="xc",
                                       name="xc0")
                    nc.sync.dma_start(xc0[:], xt_r[:, :, 0:TOK_CHUNK])
                if o["split_dma"]:
                    for i in range(n_ctiles):
                        nc.scalar.dma_start(wqkv_sb[:, i, :], wqkv_r[:, i, :])
                    nc.scalar.dma_start(wot_sb[:], wot_r[:])
                elif o["wsplit"]:
                    nc.sync.dma_start(wqkv_sb[:, :, 0:512], wqkv_r[:, :, 0:512])
                    nc.sync.dma_start(wqkv_sb[:, :, 512:768],
                                      wqkv_r[:, :, 512:768])
                    nc.sync.dma_start(wot_sb[:], wot_r[:])
                else:
                    nc.sync.dma_start(wqkv_sb[:], wqkv_r[:])
                    nc.sync.dma_start(wot_sb[:], wot_r[:])
            nc.sync.dma_start(lam_sb[:], lamneg.ap())
            make_identity(nc, ident[:])
            # 1.0 where kk <= q (partition <= free), else 0
            make_upper_triangular(nc, trimask[:], val=1.0, diag=True)
            qt_pool = ctx.enter_context(tc.tile_pool(name="qt", bufs=2))
            kt_pool = ctx.enter_context(tc.tile_pool(name="kt", bufs=2))
            ksq_pool = ctx.enter_context(tc.tile_pool(name="ksq", bufs=2))
            kscale_pool = ctx.enter_context(tc.tile_pool(name="kscale", bufs=2))
            v_pool = ctx.enter_context(tc.tile_pool(name="v", bufs=2))
            yt_pool = ctx.enter_context(tc.tile_pool(name="yt", bufs=2))
            pt_pool = ctx.enter_context(tc.tile_pool(name="pt", bufs=o["pt_bufs"]))
            y0_pool = ctx.enter_context(tc.tile_pool(name="y0", bufs=o["y0_mult"] * jpc))
            osb_pool = ctx.enter_context(tc.tile_pool(name="osb", bufs=3))
            qn_pool = ctx.enter_context(tc.tile_pool(name="qn", bufs=o["qn_bufs"]))
            sq_pool = ctx.enter_context(tc.tile_pool(name="sq", bufs=2))
            rms_pool = ctx.enter_context(tc.tile_pool(name="rms", bufs=o["rms_bufs"]))
            nproj, nst, ny = o["psum"]
            psum_proj = ctx.enter_context(
                tc.tile_pool(name="psum_proj", bufs=nproj, space="PSUM"))
            psum_st = ctx.enter_context(
                tc.tile_pool(name="psum_st", bufs=nst, space="PSUM"))
            psum_y = ctx.enter_context(
                tc.tile_pool(name="psum_y", bufs=ny, space="PSUM"))

            tr_psum = psum_st if o["tr_pool"] == "st" else psum_proj
            tr_tag = o["tr_pool"] if o["tr_pool"] == "st" else "pp"
            tr_shape = 256 if o["tr_pool"] == "st" else 512

            def pe_transpose(dst_ap, src_ap):
                trp = tr_psum.tile([P, tr_shape], bf16, tag=tr_tag,
                                   name="trp")[:, :P]
                nc.tensor.transpose(trp, src_ap, ident[:])
                nc.vector.tensor_copy(dst_ap, trp)

            def pe_transpose_y(dst_ap, src_ap):
                trp = psum_y.tile([P, 257], bf16, tag="y", name="trpy")[:, :P]
                nc.tensor.transpose(trp, src_ap, ident[:])
                nc.vector.tensor_copy(dst_ap, trp)

            def dma_transpose(dst_ap, src_ap):
                tr_eng.dma_start_transpose(out=dst_ap, in_=src_ap)

            tr_qk = pe_transpose if o["qk_tr"] == "pe" else dma_transpose
            tr_y = pe_transpose if o["y_tr"] == "pe" else dma_transpose
            if o["ytr_pool"] == "y":
                tr_y = pe_transpose_y

            for b in range(b_dim):
                qt_sb = qt_pool.tile([P, 2, t_dim], bf16, name=f"qt_b{b}", tag="qt")
                kt_sb = kt_pool.tile([P, 2, t_dim], bf16, name=f"kt_b{b}", tag="kt")
                v_sb = v_pool.tile([P, n_blocks_b, VP], bf16, name=f"v_b{b}", tag="v")
                yt_sb = yt_pool.tile([P, 2, t_dim], bf16, name=f"yt_b{b}", tag="yt")
                kscale_sb = (kscale_pool.tile([P, 2 * n_blocks_b], f32,
                                              name=f"ksc_b{b}", tag="ksc")
                             if o["k_major"] else None)

                # ================= projections =================
                RB = o["rms_batch"]
                for ch in range(t_dim // TOK_CHUNK):
                    tok0 = b * t_dim + ch * TOK_CHUNK
                    if b == 0 and ch == 0 and xc0 is not None:
                        xc = xc0
                    else:
                        xc = xc_pool.tile([P, n_ctiles, TOK_CHUNK], bf16, tag="xc")
                        if o["split_dma"] and b == 0 and ch == 0:
                            for i in range(n_ctiles):
                                nc.sync.dma_start(
                                    xc[:, i, :], xt_r[:, i, tok0:tok0 + TOK_CHUNK])
                        else:
                            nc.sync.dma_start(
                                xc[:], xt_r[:, :, tok0:tok0 + TOK_CHUNK])

                    if o["k_major"]:
                        # --- K projection straight into [d, tok] layout ---
                        # wqkv packs [q(256) | v(256) | k(256)]; lhsT slices of
                        # the k block give KT = wk_h @ x.T per d-tile (= view)
                        kssq = psum_proj.tile([P, 512], f32, tag="pp",
                                              name="kssq")[:, :8]
                        for v in range(2):
                            ktp = psum_proj.tile([P, 512], f32, tag="pp",
                                                 name="ktp")
                            for i in range(n_ctiles):
                                nc.tensor.matmul(
                                    ktp[:],
                                    wqkv_sb[:, i, 512 + v * P:512 + (v + 1) * P],
                                    xc[:, i, :],
                                    start=(i == 0), stop=(i == n_ctiles - 1))
                            ktdst = kt_sb[:, v, ch * TOK_CHUNK:(ch + 1) * TOK_CHUNK]
                            if o["ktcopy_eng"] == "act":
                                nc.scalar.copy(ktdst, ktp[:])
                            else:
                                nc.vector.tensor_copy(ktdst, ktp[:])
                            ksq = ksq_pool.tile([P, TOK_CHUNK], bf16, tag="ksq")
                            ksrc = ktdst if o["ksq_src"] == "sbuf" else ktp[:]
                            if o["ksq_eng"] == "act":
                                nc.scalar.activation(ksq[:], ksrc, AF.Square)
                            else:
                                nc.vector.tensor_tensor(ksq[:], ksrc, ksrc,
                                                        op=OP.mult)
                            for t in range(blocks_per_chunk):
                                nc.tensor.matmul(
                                    kssq[:, 2 * t + v:2 * t + v + 1],
                                    ksq[:, t * P:(t + 1) * P], ones_sb[:],
                                    start=True, stop=True)
                        # kscale = 1/sqrt(ssq + 128*eps)  (includes 1/sqrt(128))
                        ksl = kscale_sb[:, ch * 2 * blocks_per_chunk:
                                        (ch + 1) * 2 * blocks_per_chunk]
                        if o["rms_mode"] == "sqrt":
                            nc.vector.tensor_scalar(ksl, kssq[:], 1.0,
                                                    HALF * RMS_EPS, OP.mult, OP.add)
                            nc.scalar.activation(ksl, ksl, AF.Sqrt)
                            nc.vector.reciprocal(ksl, ksl)
                        else:
                            # rescale into the Newton seed's fit range, then
                            # multiply the 1/sqrt(128) back in at the end
                            km = rms_pool.tile([P, 8], f32, tag="rms", name="km")
                            nc.vector.tensor_scalar(km[:], kssq[:], 1.0 / HALF,
                                                    RMS_EPS, OP.mult, OP.add)
                            kt1 = rms_pool.tile([P, 8], f32, tag="rms", name="kt1")
                            nc.vector.tensor_tensor(kt1[:], km[:], km[:], op=OP.mult)
                            nc.vector.tensor_scalar(ksl, km[:], -1.47991565,
                                                    2.07556761, OP.mult, OP.add)
                            nc.vector.scalar_tensor_tensor(
                                ksl, kt1[:], 0.41306651, ksl, op0=OP.mult,
                                op1=OP.add)
                            nc.vector.tensor_scalar_max(ksl, ksl, 0.05)
                            for _ in range(2):
                                nc.vector.tensor_tensor(kt1[:], ksl, ksl,
                                                        op=OP.mult)
                                nc.vector.scalar_tensor_tensor(
                                    kt1[:], kt1[:], -0.5, km[:], op0=OP.mult,
                                    op1=OP.mult)
                                nc.vector.tensor_scalar(kt1[:], kt1[:], 1.0, 1.5,
                                                        OP.mult, OP.add)
                                nc.vector.tensor_tensor(ksl, ksl, kt1[:],
                                                        op=OP.mult)
                            nc.vector.tensor_scalar_mul(ksl, ksl, inv_sqrt_half)

                    NH = 2 if o["k_major"] else 4   # rms'd halves per block
                    for tl0 in range(0, blocks_per_chunk, RB):
                        group = []   # (tb, qkv0, qkv1)
                        rmsg = rms_pool.tile([P, NH * RB], f32, tag="rms")
                        for g in range(RB):
                            tl = tl0 + g
                            tb = ch * blocks_per_chunk + tl
                            if o["k_major"]:
                                # one bank: [q1 q2 | v]
                                qv = psum_proj.tile([P, 512], f32, tag="pp",
                                                    name="qv")
                                first_blk = (b == 0 and ch == 0 and tl == 0
                                             and xcmini is not None)
                                for i in range(n_ctiles):
                                    lhsT = (xcmini[:, i, :] if first_blk
                                            else xc[:, i, tl * P:(tl + 1) * P])
                                    nc.tensor.matmul(
                                        qv[:], lhsT,
                                        wqkv_sb[:, i, 0:512],
                                        start=(i == 0), stop=(i == n_ctiles - 1))
                                qkv0, qkv1 = qv, qv
                                halves = [qv[:, 0:128], qv[:, 128:256]]
                            else:
                                qkv0 = psum_proj.tile([P, 512], f32, tag="pp",
                                                      name="qkv0")[:, :384]
                                qkv1 = psum_proj.tile([P, 512], f32, tag="pp",
                                                      name="qkv1")[:, :384]
                                for i in range(n_ctiles):
                                    lhsT = xc[:, i, tl * P:(tl + 1) * P]
                                    nc.tensor.matmul(qkv0, lhsT, wqkv_sb[:, i, 0:384],
                                                     start=(i == 0), stop=(i == n_ctiles - 1))
                                    nc.tensor.matmul(qkv1, lhsT, wqkv_sb[:, i, 384:768],
                                                     start=(i == 0), stop=(i == n_ctiles - 1))
                                # layout: qkv0 = [q1 q2 k1], qkv1 = [k2 v]
                                halves = [qkv0[:, 0:128], qkv0[:, 128:256],
                                          qkv0[:, 256:384], qkv1[:, 0:128]]
                            for j, h in enumerate(halves):
                                sq = sq_pool.tile([P, P], bf16, tag="sq", name="sq")
                                nc.scalar.activation(
                                    sq[:], h, AF.Square,
                                    accum_out=rmsg[:, NH * g + j:NH * g + j + 1])
                            group.append((tb, qkv0, qkv1, halves))
                        # rms: ssq -> 1/sqrt(ssq/128 + eps), batched over the group
                        nc.vector.tensor_scalar(rmsg[:], rmsg[:], 1.0 / HALF,
                                                RMS_EPS, OP.mult, OP.add)
                        if o["rms_mode"] == "sqrt":
                            nc.scalar.activation(rmsg[:], rmsg[:], AF.Sqrt)
                            nc.vector.reciprocal(rmsg[:], rmsg[:])
                        else:
                            # DVE-only rsqrt: quadratic seed + 2 Newton steps.
                            # m concentrates near 0.8 for rms-normed randn
                            # inputs; seed is a least-squares quadratic fit of
                            # rsqrt on m in [0.3, 2.0], clamped for safety.
                            m = rmsg
                            yv = rms_pool.tile([P, NH * RB], f32, tag="rms")
                            t1 = rms_pool.tile([P, NH * RB], f32, tag="rms")
                            nc.vector.tensor_tensor(t1[:], m[:], m[:], op=OP.mult)
                            nc.vector.tensor_scalar(yv[:], m[:], -1.47991565, 2.07556761,
                                                    OP.mult, OP.add)
                            nc.vector.scalar_tensor_tensor(
                                yv[:], t1[:], 0.41306651, yv[:], op0=OP.mult,
                                op1=OP.add)
                            nc.vector.tensor_scalar_max(yv[:], yv[:], 0.05)
                            for _ in range(2):
                                nc.vector.tensor_tensor(t1[:], yv[:], yv[:],
                                                        op=OP.mult)
                                nc.vector.scalar_tensor_tensor(
                                    t1[:], t1[:], -0.5, m[:], op0=OP.mult,
                                    op1=OP.mult)
                                nc.vector.tensor_scalar(t1[:], t1[:], 1.0, 1.5,
                                                        OP.mult, OP.add)
                                nc.vector.tensor_tensor(yv[:], yv[:], t1[:],
                                                        op=OP.mult)
                            rmsg = yv
                        dests = [(qt_sb, 0), (qt_sb, 1), (kt_sb, 0), (kt_sb, 1)]
                        for g, (tb, qkv0, qkv1, halves) in enumerate(group):
                            for j, h in enumerate(halves):
                                qn = qn_pool.tile([P, P], bf16, tag="qn")
                                nc.vector.tensor_scalar_mul(
                                    qn[:], h, rmsg[:, NH * g + j:NH * g + j + 1])
                                dst, view = dests[j]
                                tr_qk(dst[:, view, tb * P:(tb + 1) * P], qn[:])
                            # V (+ ones column for the softmax denominator)
                            vsrc = (qkv1[:, 256:512] if o["k_major"]
                                    else qkv1[:, 128:384])
                            if o["vcopy"] == "act":
                                nc.scalar.copy(v_sb[:, tb, 0:256], vsrc)
                            else:
                                nc.vector.tensor_copy(v_sb[:, tb, 0:256], vsrc)
                            nc.vector.memset(v_sb[:, tb, 256:257], 1.0)

                # ================= attention =================
                cqi_order = (range(n_qchunks) if o["chunk_order"] == "asc"
                             else range(n_qchunks - 1, -1, -1))
                for cqi in cqi_order:
                    q0 = cqi * QCH
                    jmax = jpc * cqi + (jpc - 1)   # top kk-tile in this chunk
                    y0s = []
                    for v in range(2):
                        ys = [psum_y.tile([P, 257], f32, tag="y", name="ys")
                              for _ in range(jpc)]
                        for i in range(jmax + 1):
                            # jj0: first valid j-slot for this row (causal)
                            jj0 = max(0, i - jpc * cqi) if o["narrow_top"] else 0
                            w = QCH - jj0 * P
                            st = psum_st.tile([P, QCH], f32, tag="st",
                                              name="st")[:, :w]
                            nc.tensor.matmul(
                                st[:], kt_sb[:, v, i * P:(i + 1) * P],
                                qt_sb[:, v, q0 + jj0 * P:q0 + QCH],
                                start=True, stop=True)
                            pt = pt_pool.tile([P, QCH], bf16, tag="pt", name="pt")[:, :w]
                            if o["k_major"]:
                                nc.scalar.activation(
                                    pt[:], st[:], AF.Exp,
                                    scale=kscale_sb[:, 2 * i + v:2 * i + v + 1])
                            else:
                                nc.scalar.activation(pt[:], st[:], AF.Exp,
                                                     scale=inv_sqrt_half)
                            dj = i - jpc * cqi    # diagonal j-slot if >= 0
                            if dj >= 0:
                                nc.vector.tensor_tensor(
                                    pt[:, (dj - jj0) * P:(dj - jj0 + 1) * P],
                                    pt[:, (dj - jj0) * P:(dj - jj0 + 1) * P],
                                    trimask[:], op=OP.mult)
                            for jj in range(jj0, jpc):
                                j = jpc * cqi + jj
                                if i > j:
                                    continue
                                nc.tensor.matmul(
                                    ys[jj][:],
                                    pt[:, (jj - jj0) * P:(jj - jj0 + 1) * P],
                                    v_sb[:, i, 0:257],
                                    start=(i == 0), stop=(i == j))
                        # epilogue for this view
                        for jj in range(jpc):
                            j = jpc * cqi + jj
                            inv = rms_pool.tile([P, 1], f32, tag="inv")
                            nc.vector.reciprocal(inv[:], ys[jj][:, 256:257])
                            if v == 0:
                                y0 = y0_pool.tile([P, 256], f32, tag="y0")
                                nc.vector.tensor_scalar_mul(
                                    y0[:], ys[jj][:, 0:256], inv[:])
                                y0s.append(y0)
                            else:
                                sc2 = rms_pool.tile([P, 1], f32, tag="inv")
                                nc.vector.tensor_tensor(
                                    sc2[:], inv[:], lam_sb[:], op=OP.mult)
                                yf = qn_pool.tile([P, 256], bf16, tag="yf")
                                nc.vector.scalar_tensor_tensor(
                                    yf[:], ys[jj][:, 0:256], sc2[:], y0s[jj][:],
                                    op0=OP.mult, op1=OP.add)
                                for e in range(2):
                                    tr_y(yt_sb[:, e, j * P:(j + 1) * P],
                                         yf[:, e * P:(e + 1) * P])

                # ================= output projection (partial) =================
                for tb in range(n_blocks_b):
                    row0 = b * t_dim + tb * P
                    merged = o["osb_merge"] and not (
                        o["tail_split"] and b == b_dim - 1
                        and tb == n_blocks_b - 1)
                    orow = (osb_pool.tile([P, c_dim], f32, tag="orow",
                                          name="orow")
                            if merged else None)
                    for cc in range(c_dim // 512):
                        op_ps = psum_proj.tile([P, 512], f32, tag="pp", name="ops")
                        for e in range(2):
                            nc.tensor.matmul(
                                op_ps[:], yt_sb[:, e, tb * P:(tb + 1) * P],
                                wot_sb[:, e, cc * 512:(cc + 1) * 512],
                                start=(e == 0), stop=(e == 1))
                        osb = (orow[:, cc * 512:(cc + 1) * 512] if merged
                               else osb_pool.tile([P, 512], f32, tag="osb"))
                        oc = o["oproj_copy"]
                        if oc == "alt":
                            oc = "act" if (tb + cc) % 2 == 0 else "dve"
                        if oc == "act":
                            nc.scalar.copy(osb, op_ps[:])
                        else:
                            nc.vector.tensor_copy(osb, op_ps[:])
                        if not merged:
                            out_eng = nc.sync
                            if o["out_dma_alt"] and (tb + cc) % 2 == 1:
                                out_eng = nc.scalar
                            out_eng.dma_start(
                                out.ap()[row0:row0 + P,
                                         cc * 512:(cc + 1) * 512], osb)
                    if merged:
                        out_eng = nc.sync
                        if o["out_dma_alt"] and tb % 2 == 1:
                            out_eng = nc.scalar
                        out_eng.dma_start(
                            out.ap()[row0:row0 + P, :], orow[:])
    nc.compile()
    return nc


_NC_CACHE = {}
TRACE = False        # set True (e.g. from test.py) to capture an NTFF profile
LAST_RESULT = None   # BassKernelResults of the most recent run


def _get_nc(c_dim, t_dim, b_dim):
    key = (c_dim, t_dim, b_dim)
    if key not in _NC_CACHE:
        _NC_CACHE[key] = build_nc(c_dim, t_dim, b_dim)
    return _NC_CACHE[key]


def prep_inputs(x, wq, wk, wv, wo, lq1, lk1, lq2, lk2, k_major=None):
    """Host-side prep: per-core input maps."""
    import ml_dtypes

    if k_major is None:
        k_major = DEFAULT_OPTS["k_major"]

    bf16 = ml_dtypes.bfloat16
    b_dim, t_dim, c_dim = x.shape

    lam1 = np.exp(np.sum(lq1.astype(np.float64) * lk1.astype(np.float64)))
    lam2 = np.exp(np.sum(lq2.astype(np.float64) * lk2.astype(np.float64)))
    lam_full = np.float32(lam1 - lam2 + LAMBDA_INIT)

    xtb = np.ascontiguousarray(
        x.reshape(b_dim * t_dim, c_dim).T).astype(bf16)
    lamneg = np.full((P, 1), -lam_full, dtype=np.float32)

    in_maps = []
    for h in range(N_CORES):
        sl = slice(h * HEAD_DIM, (h + 1) * HEAD_DIM)
        parts = ([wq[sl].T, wv[sl].T, wk[sl].T] if k_major
                 else [wq[sl].T, wk[sl].T, wv[sl].T])
        wqkv_h = np.concatenate(parts, axis=1).astype(bf16)
        wqkv_h = np.ascontiguousarray(wqkv_h)
        wot_h = np.ascontiguousarray(
            (wo[:, sl] * (1.0 - LAMBDA_INIT)).T).astype(bf16)
        in_maps.append({
            "xt": xtb, "wqkv": wqkv_h, "wot": wot_h, "lamneg": lamneg,
        })
    return in_maps


_FN_CACHE = {}


def _get_callable(nc):
    """Build (once) a reusable jitted shard_map callable for this module —
    mirrors bass2jax.run_bass_via_pjrt's multi-core path, but cached so
    repeat kernel() calls skip retracing."""
    if id(nc) in _FN_CACHE:
        return _FN_CACHE[id(nc)]
    import jax
    from jax.sharding import Mesh, PartitionSpec, NamedSharding
    from jax.experimental.shard_map import shard_map
    import concourse.mybir as mybir
    import concourse.bass2jax as b2j

    b2j.install_neuronx_cc_hook()
    pname = nc.partition_id_tensor.name if nc.partition_id_tensor else None
    in_names, out_names, out_avals, zero_shapes = [], [], [], []
    for alloc in nc.m.functions[0].allocations:
        if not isinstance(alloc, mybir.MemoryLocationSet):
            continue
        name = alloc.memorylocations[0].name
        if alloc.kind == "ExternalInput":
            if name != pname:
                in_names.append(name)
        elif alloc.kind == "ExternalOutput":
            out_names.append(name)
            shape = tuple(alloc.tensor_shape)
            dtype = mybir.dt.np(alloc.dtype)
            out_avals.append(jax.core.ShapedArray(shape, dtype))
            zero_shapes.append((shape, dtype))
    n_params = len(in_names)
    all_in = in_names + out_names
    if pname is not None:
        all_in = all_in + [pname]

    def _body(*args):
        operands = list(args)
        if pname is not None:
            operands.append(b2j.partition_id_tensor())
        return tuple(b2j._bass_exec_p.bind(
            *operands,
            out_avals=tuple(out_avals),
            in_names=tuple(all_in),
            out_names=tuple(out_names),
            lowering_input_output_aliases=(),
            sim_require_finite=True,
            sim_require_nnan=True,
            nc=nc,
        ))

    devices = jax.devices()[:N_CORES]
    mesh = Mesh(np.asarray(devices), ("core",))
    nio = n_params + len(out_names)
    fn = jax.jit(shard_map(_body, mesh=mesh,
                           in_specs=(PartitionSpec("core"),) * nio,
                           out_specs=(PartitionSpec("core"),) * len(out_names),
                           check_rep=False),
                 donate_argnums=tuple(range(n_params, nio)), keep_unused=True)
    sh = NamedSharding(mesh, PartitionSpec("core"))
    entry = (fn, in_names, out_names, zero_shapes, sh)
    _FN_CACHE[id(nc)] = entry
    return entry


def kernel(x, wq, wk, wv, wo, lq1, lk1, lq2, lk2):
    b_dim, t_dim, c_dim = x.shape
    in_maps = prep_inputs(x, wq, wk, wv, wo, lq1, lk1, lq2, lk2)
    nc = _get_nc(c_dim, t_dim, b_dim)

    try:
        import jax
        fn, in_names, out_names, zero_shapes, sh = _get_callable(nc)
        concat_in = [
            np.concatenate([np.asarray(in_maps[c][n]) for c in range(N_CORES)],
                           axis=0) for n in in_names]
        concat_zeros = [np.zeros((N_CORES * s[0], *s[1:]), d)
                        for s, d in zero_shapes]
        dev_in = [jax.device_put(a, sh) for a in concat_in]
        dev_zero = [jax.device_put(a, sh) for a in concat_zeros]
        outs = fn(*dev_in, *dev_zero)
        arr = np.asarray(outs[out_names.index("out")])
        acc = arr.reshape(N_CORES, b_dim * t_dim, c_dim).sum(
            axis=0, dtype=np.float32)
    except Exception:
        from concourse.bass_utils import run_bass_kernel_spmd
        res = run_bass_kernel_spmd(nc, in_maps, list(range(N_CORES)),
                                   trace=TRACE)
        global LAST_RESULT
        LAST_RESULT = res
        acc = np.zeros((b_dim * t_dim, c_dim), dtype=np.float32)
        for h in range(N_CORES):
            acc += res.results[h]["out"]
    return acc.reshape(b_dim, t_dim, c_dim)



# revision 21
# speedup vs baseline: 1.2199x; 1.2199x over previous
"""Differential attention (B=2, T=2048, C=2048, 8 heads x 256) on 8 trn2 cores.

Sharding: tensor-parallel over the 8 effective heads — core h computes head h's
projections + attention and a partial output projection; host sums bf16
partials in f32.

Per-core pipeline (fp8 DoubleRow matmuls with hi/lo error compensation for the
projections and output projection; bf16 for QK^T and PV):
  xh/xl    [C, B*T]  fp8 e4m3 of x.T*16 (hi) and its residual (lo)
  wqvh/l   [C, 512]  head slice of [wq|wv].T * 1024, fp8 hi/lo
  wkh/l    [128, 16, 256]  head slice of wk.T * 1024 (pre-swizzled), fp8 hi/lo
  woth/l   [256, C]  head slice of wo.T * 0.2 * 4096, fp8 hi/lo
  A @ B is computed as Ah@Bh + Al@Bh + Ah@Bl, each pair of 128-contraction
  tiles packed into one DoubleRow matmul (0.5 cycles/row) -> 0.75x the bf16
  cost with ~bf16 accuracy. Scales are powers of two, folded into the existing
  descale points (rms-norm Rsqrt scale, V/KT psum evacuation, out-proj copy).

Emission is interleaved into units (proj chunk / attention q-chunk / out-proj
block pair) so the in-order PE stream always has ready matmuls; the PE
transposes trailing each unit are deferred one unit ("pending") so their
ACT/DVE producer chains overlap earlier matmul groups.

Attention math: scores computed transposed (S.T[kk,q] = K_tile.T @ Q), exp on
ACT with per-partition kscale, P.T tiles feed PV as lhsT, ones-column on V
gives the softmax denominator. Causal blocks skipped; diagonal masked
multiplicatively post-exp.
"""

import math
from contextlib import ExitStack

import numpy as np

# ---- problem constants (hardcoded per the harness contract) ----
B = 2
T = 2048
C = 2048
N_HEAD = 8
HEAD_DIM = 256
HALF = 128
LAMBDA_INIT = 0.8
RMS_EPS = 1.1920929e-07
N_CORES = 8

P = 128          # partitions
TOK_CHUNK = 512  # projection tok chunk (DMA granularity)

# fp8 scale plan (all powers of two; see module docstring)
SX = 16.0        # x -> x*SX before e4m3
SW = 1024.0      # wq|wv|wk -> *SW
SWO = 4096.0     # wo*0.2 -> *SWO
SY = 8.0         # y -> y*SY at the hi/lo split
QV_DESCALE = 1.0 / (SX * SW)              # 2^-14, applied at V/KT evacuation
RMS_SSQ_SCALE = QV_DESCALE * QV_DESCALE   # 2^-28, folded into rms Rsqrt scale
OUT_DESCALE = 1.0 / (SY * SWO)            # 2^-15, applied at out-proj copy

DEFAULT_OPTS = dict(
    att_chunk=256,       # attention q-chunk width (256 or 512)
    qk_tr="pe",          # "pe" | "dma": Q/K transpose path
    oproj_copy="alt",    # out-proj PSUM->SBUF evacuation: "act"|"dve"|"alt"
    psum=(3, 3, 2),      # banks: (proj, st, y) — must sum to <= 8
    pt_bufs=5,           # P.T tile double-buffer depth
    xc_bufs=3,           # x chunk prefetch depth
    vcopy="dve",         # "act" | "dve": V PSUM->SBUF descale-copy engine
    osb_merge=True,      # one output-store DMA per tok block (vs per c-chunk)
    narrow_top=True,     # compute only the valid half of the top causal row
    tr_pool="st",        # "st" | "pp": PSUM pool used by PE transposes
    tail_split=False,    # stream the final block's stores per c-chunk
    qn_bufs=6,
    y0_mult=2,
    ksq_eng="dve",       # "act" | "dve": engine computing k^2
    ktcopy_eng="dve",    # "act" | "dve": engine evacuating KT psum
    ytr_pool="y",        # "st" | "y": PSUM pool for the y transposes
    out_queue="sync",    # "sync" | "scalar": HWDGE queue for output stores
    prefetch=6,          # unit lookahead for x chunk DMA issue
)


PHASES = []      # (label, first_instruction_id) marks recorded during build


def build_nc(c_dim, t_dim, b_dim, **opts):
    """Build the per-core Bass module. All shapes in tokens/channels."""
    import concourse.mybir as mybir
    import concourse.tile as tile
    from concourse import bacc
    from concourse.masks import make_identity, make_upper_triangular

    o = dict(DEFAULT_OPTS)
    o.update(opts)
    QCH = o["att_chunk"]
    jpc = QCH // P  # j-blocks per attention chunk

    dt = mybir.dt
    f32 = dt.float32
    bf16 = dt.bfloat16
    f8 = dt.float8e4
    AF = mybir.ActivationFunctionType
    OP = mybir.AluOpType
    DR = mybir.MatmulPerfMode.DoubleRow

    n_ctiles = c_dim // P            # contraction tiles over C
    n_cpairs = n_ctiles // 2         # DoubleRow c-tile pairs
    ntok = b_dim * t_dim             # total token rows
    n_blocks_b = t_dim // P          # 128-tok blocks per batch
    n_qchunks = t_dim // QCH         # attention q chunks per batch
    blocks_per_chunk = TOK_CHUNK // P
    n_chunks = t_dim // TOK_CHUNK
    VP = 272                         # V tile pitch (256 vals + 1 ones + pad)

    nc = bacc.Bacc()

    def mark(label):
        PHASES.append((label, nc.next_id()))

    xh = nc.declare_dram_parameter("xh", [c_dim, ntok], f8, isOutput=False)
    xl = nc.declare_dram_parameter("xl", [c_dim, ntok], f8, isOutput=False)
    wqvh = nc.declare_dram_parameter("wqvh", [c_dim, 512], f8, isOutput=False)
    wqvl = nc.declare_dram_parameter("wqvl", [c_dim, 512], f8, isOutput=False)
    # wk arrives pre-swizzled: [128 partitions, 16 ctiles, 256] row-major so a
    # partition's whole slab is one contiguous 4KB descriptor.
    wkh = nc.declare_dram_parameter("wkh", [P, n_ctiles * 256], f8, isOutput=False)
    wkl = nc.declare_dram_parameter("wkl", [P, n_ctiles * 256], f8, isOutput=False)
    woth = nc.declare_dram_parameter("woth", [HEAD_DIM, c_dim], f8, isOutput=False)
    wotl = nc.declare_dram_parameter("wotl", [HEAD_DIM, c_dim], f8, isOutput=False)
    lamneg = nc.declare_dram_parameter("lamneg", [P, 1], f32, isOutput=False)
    out = nc.declare_dram_parameter("out", [ntok, c_dim], bf16, isOutput=True)

    xh_r = xh.ap().rearrange("(i p) t -> p i t", p=P)      # [128, n_ctiles, ntok]
    xl_r = xl.ap().rearrange("(i p) t -> p i t", p=P)
    wqvh_r = wqvh.ap().rearrange("(i p) n -> p i n", p=P)  # [128, n_ctiles, 512]
    wqvl_r = wqvl.ap().rearrange("(i p) n -> p i n", p=P)
    wkh_r = wkh.ap().rearrange("p (i n) -> p i n", n=256)  # [128, n_ctiles, 256]
    wkl_r = wkl.ap().rearrange("p (i n) -> p i n", n=256)
    woth_r = woth.ap().rearrange("(e p) n -> p e n", p=P)  # [128, 2, c_dim]
    wotl_r = wotl.ap().rearrange("(e p) n -> p e n", p=P)

    with tile.TileContext(nc) as tc:
        with ExitStack() as ctx:
            # ---- persistent SBUF ----
            const_pool = ctx.enter_context(tc.tile_pool(name="const", bufs=1))
            wqvh_sb = const_pool.tile([P, n_ctiles, 512], f8, name="wqvh_sb")
            wqvl_sb = const_pool.tile([P, n_ctiles, 512], f8, name="wqvl_sb")
            wkh_sb = const_pool.tile([P, n_ctiles, 256], f8, name="wkh_sb")
            wkl_sb = const_pool.tile([P, n_ctiles, 256], f8, name="wkl_sb")
            woth_sb = const_pool.tile([P, 2, c_dim], f8, name="woth_sb")
            wotl_sb = const_pool.tile([P, 2, c_dim], f8, name="wotl_sb")
            lam_sb = const_pool.tile([P, 1], f32, name="lam_sb")
            ident = const_pool.tile([P, P], bf16, name="ident")
            trimask = const_pool.tile([P, P], bf16, name="trimask")
            ones_sb = const_pool.tile([P, 1], bf16, name="ones_sb")
            nc.vector.memset(ones_sb[:], 1.0)

            xch_pool = ctx.enter_context(tc.tile_pool(name="xch", bufs=o["xc_bufs"]))
            xcl_pool = ctx.enter_context(tc.tile_pool(name="xcl", bufs=o["xc_bufs"]))

            # ---- ramp: first matmul's operands first; weights on the ACT
            # queue, x chunks + output stores on the SP queue ----
            nc.scalar.dma_start(wkh_sb[:], wkh_r[:])
            xc0h = xch_pool.tile([P, n_ctiles, TOK_CHUNK], f8, tag="xch",
                                 name="xc0h")
            for pc in range(4):     # stream the very first chunk in quarters
                nc.sync.dma_start(xc0h[:, 4 * pc:4 * pc + 4, :],
                                  xh_r[:, 4 * pc:4 * pc + 4, 0:TOK_CHUNK])
            nc.scalar.dma_start(wkl_sb[:], wkl_r[:])
            xc0l = xcl_pool.tile([P, n_ctiles, TOK_CHUNK], f8, tag="xcl",
                                 name="xc0l")
            for pc in range(4):
                nc.sync.dma_start(xc0l[:, 4 * pc:4 * pc + 4, :],
                                  xl_r[:, 4 * pc:4 * pc + 4, 0:TOK_CHUNK])
            nc.scalar.dma_start(wqvh_sb[:], wqvh_r[:])
            nc.scalar.dma_start(wqvl_sb[:], wqvl_r[:])
            nc.scalar.dma_start(lam_sb[:], lamneg.ap())
            make_identity(nc, ident[:])
            # 1.0 where kk <= q (partition <= free), else 0
            make_upper_triangular(nc, trimask[:], val=1.0, diag=True)

            qt_pool = ctx.enter_context(tc.tile_pool(name="qt", bufs=2))
            kt_pool = ctx.enter_context(tc.tile_pool(name="kt", bufs=2))
            ksq_pool = ctx.enter_context(tc.tile_pool(name="ksq", bufs=2))
            kscale_pool = ctx.enter_context(tc.tile_pool(name="kscale", bufs=2))
            v_pool = ctx.enter_context(tc.tile_pool(name="v", bufs=2))
            yth_pool = ctx.enter_context(tc.tile_pool(name="yth", bufs=5))
            ytl_pool = ctx.enter_context(tc.tile_pool(name="ytl", bufs=5))
            pt_pool = ctx.enter_context(tc.tile_pool(name="pt", bufs=o["pt_bufs"]))
            y0_pool = ctx.enter_context(tc.tile_pool(name="y0", bufs=o["y0_mult"] * jpc))
            osb_pool = ctx.enter_context(tc.tile_pool(name="osb", bufs=3))
            qn_pool = ctx.enter_context(tc.tile_pool(name="qn", bufs=o["qn_bufs"]))
            sq_pool = ctx.enter_context(tc.tile_pool(name="sq", bufs=2))
            qcp_pool = ctx.enter_context(tc.tile_pool(name="qcp", bufs=8))
            rms_pool = ctx.enter_context(tc.tile_pool(name="rms", bufs=4))
            nproj, nst, ny = o["psum"]
            psum_proj = ctx.enter_context(
                tc.tile_pool(name="psum_proj", bufs=nproj, space="PSUM"))
            psum_st = ctx.enter_context(
                tc.tile_pool(name="psum_st", bufs=nst, space="PSUM"))
            psum_y = ctx.enter_context(
                tc.tile_pool(name="psum_y", bufs=ny, space="PSUM"))

            tr_psum = psum_st if o["tr_pool"] == "st" else psum_proj
            tr_tag = o["tr_pool"] if o["tr_pool"] == "st" else "pp"
            tr_shape = 256 if o["tr_pool"] == "st" else 512

            def pe_transpose(dst_ap, src_ap):
                trp = tr_psum.tile([P, tr_shape], bf16, tag=tr_tag,
                                   name="trp")[:, :P]
                nc.tensor.transpose(trp, src_ap, ident[:])
                nc.vector.tensor_copy(dst_ap, trp)

            tr_qk = pe_transpose

            # ---------------- per-batch tile state ----------------
            bt = {}

            def batch_tiles(b):
                if b not in bt:
                    bt[b] = dict(
                        qt=qt_pool.tile([P, 2, t_dim], bf16, name=f"qt_b{b}", tag="qt"),
                        kt=kt_pool.tile([P, 2, t_dim], bf16, name=f"kt_b{b}", tag="kt"),
                        v=v_pool.tile([P, n_blocks_b, VP], bf16, name=f"v_b{b}", tag="v"),
                        ksc=kscale_pool.tile([P, 2 * n_blocks_b], f32,
                                             name=f"ksc_b{b}", tag="ksc"),
                    )
                return bt[b]

            pending = []     # deferred emitters (PE transposes of last unit)

            def flush_pending():
                for fn in pending:
                    fn()
                pending.clear()

            # x chunk DMA management (prefetched ahead of the unit stream)
            xc_tiles = {(0, 0): (xc0h, xc0l)}
            ytp = {}     # (b, attn chunk) -> per-chunk (yth, ytl) fp8 tiles

            def issue_xc(b, ch):
                if (b, ch) in xc_tiles:
                    return
                tok0 = b * t_dim + ch * TOK_CHUNK
                xch = xch_pool.tile([P, n_ctiles, TOK_CHUNK], f8, tag="xch")
                xcl = xcl_pool.tile([P, n_ctiles, TOK_CHUNK], f8, tag="xcl")
                nc.sync.dma_start(xch[:], xh_r[:, :, tok0:tok0 + TOK_CHUNK])
                nc.sync.dma_start(xcl[:], xl_r[:, :, tok0:tok0 + TOK_CHUNK])
                xc_tiles[(b, ch)] = (xch, xcl)

            # ---------------- unit emitters ----------------
            def emit_proj(b, ch):
                mark(f"b{b}_proj_ch{ch}")
                t_ = batch_tiles(b)
                kt_sb, qt_sb, v_sb, ksc_sb = t_["kt"], t_["qt"], t_["v"], t_["ksc"]
                xch, xcl = xc_tiles.pop((b, ch))

                # --- K projection straight into [d, tok] layout ---
                ksqs = []
                for v in range(2):
                    ktp = psum_proj.tile([P, 512], f32, tag="pp", name="ktp")
                    idx = 0
                    for X, W in ((xch, wkh_sb), (xch, wkl_sb), (xcl, wkh_sb)):
                        for pi in range(n_cpairs):
                            nc.tensor.matmul(
                                ktp[:],
                                W[:, 2 * pi:2 * pi + 2, v * P:(v + 1) * P],
                                X[:, 2 * pi:2 * pi + 2, :],
                                start=(idx == 0),
                                stop=(idx == 3 * n_cpairs - 1),
                                perf_mode=DR)
                            idx += 1
                    ktdst = kt_sb[:, v, ch * TOK_CHUNK:(ch + 1) * TOK_CHUNK]
                    if o["ktcopy_eng"] == "act":
                        nc.scalar.activation(ktdst, ktp[:], AF.Copy,
                                             scale=QV_DESCALE)
                    else:
                        nc.vector.tensor_scalar_mul(ktdst, ktp[:], QV_DESCALE)
                    ksq = ksq_pool.tile([P, TOK_CHUNK], bf16, tag="ksq")
                    if o["ksq_eng"] == "act":
                        nc.scalar.activation(ksq[:], ktdst, AF.Square)
                    else:
                        nc.vector.tensor_tensor(ksq[:], ktdst, ktdst, op=OP.mult)
                    ksqs.append(ksq)
                    if v == 0:
                        flush_pending()

                # --- q/v projection per 128-tok block ---
                # sq accumulates the whole chunk's ssq into one [P,8] tile; a
                # descaled q copy (qcp) releases the psum early. The Newton
                # rsqrt (GPSIMD), qn muls and PE transposes are all deferred
                # to the next unit's flush point.
                rmsg = rms_pool.tile([P, 8], f32, tag="rms")
                qcps = []
                for tl in range(blocks_per_chunk):
                    tb = ch * blocks_per_chunk + tl
                    qv = psum_proj.tile([P, 512], f32, tag="pp", name="qv")
                    idx = 0
                    for X, W in ((xch, wqvh_sb), (xcl, wqvh_sb), (xch, wqvl_sb)):
                        for pi in range(n_cpairs):
                            nc.tensor.matmul(
                                qv[:],
                                X[:, 2 * pi:2 * pi + 2, tl * P:(tl + 1) * P],
                                W[:, 2 * pi:2 * pi + 2, 0:512],
                                start=(idx == 0),
                                stop=(idx == 3 * n_cpairs - 1),
                                perf_mode=DR)
                            idx += 1
                    for j in range(2):
                        sq = sq_pool.tile([P, P], bf16, tag="sq", name="sq")
                        nc.scalar.activation(sq[:], qv[:, j * P:(j + 1) * P],
                                             AF.Square,
                                             accum_out=rmsg[:, 2 * tl + j:
                                                            2 * tl + j + 1])
                    qcp = qcp_pool.tile([P, 256], bf16, tag="qcp")
                    nc.vector.tensor_scalar_mul(qcp[:], qv[:, 0:256],
                                                QV_DESCALE)
                    qcps.append(qcp)
                    # V (+ ones column for the softmax denominator)
                    if o["vcopy"] == "act":
                        nc.scalar.activation(v_sb[:, tb, 0:256], qv[:, 256:512],
                                             AF.Copy, scale=QV_DESCALE)
                    else:
                        nc.vector.tensor_scalar_mul(v_sb[:, tb, 0:256],
                                                    qv[:, 256:512], QV_DESCALE)
                    nc.vector.memset(v_sb[:, tb, 256:257], 1.0)

                # rmsg -> 2^14/rms (the 2^14 re-scale cancels qcp's descale...
                # qcp holds true q, so use plain 1/rms): Newton rsqrt on GPSIMD
                nc.vector.tensor_scalar(rmsg[:], rmsg[:],
                                        RMS_SSQ_SCALE / HALF,
                                        RMS_EPS, OP.mult, OP.add)
                yv = rms_pool.tile([P, 8], f32, tag="rms")
                t1 = rms_pool.tile([P, 8], f32, tag="rms")
                nc.vector.tensor_tensor(t1[:], rmsg[:], rmsg[:], op=OP.mult)
                nc.vector.tensor_scalar(yv[:], rmsg[:], -1.47991565,
                                        2.07556761, OP.mult, OP.add)
                nc.vector.scalar_tensor_tensor(
                    yv[:], t1[:], 0.41306651, yv[:], op0=OP.mult, op1=OP.add)
                nc.vector.tensor_scalar_max(yv[:], yv[:], 0.05)
                for _ in range(2):
                    nc.vector.tensor_tensor(t1[:], yv[:], yv[:], op=OP.mult)
                    nc.vector.scalar_tensor_tensor(
                        t1[:], t1[:], -0.5, rmsg[:], op0=OP.mult, op1=OP.mult)
                    nc.vector.tensor_scalar(t1[:], t1[:], 1.0, 1.5,
                                            OP.mult, OP.add)
                    nc.vector.tensor_tensor(yv[:], yv[:], t1[:], op=OP.mult)

                def defer_fn(ch=ch, qcps=qcps, yv=yv):
                    for tl in range(blocks_per_chunk):
                        tb = ch * blocks_per_chunk + tl
                        for j in range(2):
                            qn = qn_pool.tile([P, P], bf16, tag="qn")
                            nc.vector.tensor_scalar_mul(
                                qn[:], qcps[tl][:, j * P:(j + 1) * P],
                                yv[:, 2 * tl + j:2 * tl + j + 1])
                            tr_qk(qt_sb[:, j, tb * P:(tb + 1) * P], qn[:])
                pending.append(defer_fn)

                # --- kssq matmuls + kscale (per-block k rms) ---
                kssq = psum_proj.tile([P, 512], f32, tag="pp",
                                      name="kssq")[:, :8]
                for v in range(2):
                    for t in range(blocks_per_chunk):
                        nc.tensor.matmul(
                            kssq[:, 2 * t + v:2 * t + v + 1],
                            ksqs[v][:, t * P:(t + 1) * P], ones_sb[:],
                            start=True, stop=True)
                ksl = ksc_sb[:, ch * 2 * blocks_per_chunk:
                             (ch + 1) * 2 * blocks_per_chunk]
                # kscale = 1/sqrt(ssq + 128*eps) == 1/(sqrt(128)*rms_k);
                # DVE-only Newton rsqrt on msq (fit range ~[0.3, 2])
                km = rms_pool.tile([P, 8], f32, tag="rms", name="km")
                # DVE for the psum read (GPSIMD cannot access PSUM)
                nc.vector.tensor_scalar(km[:], kssq[:], 1.0 / HALF,
                                        RMS_EPS, OP.mult, OP.add)
                kt1 = rms_pool.tile([P, 8], f32, tag="rms", name="kt1")
                nc.vector.tensor_tensor(kt1[:], km[:], km[:], op=OP.mult)
                nc.vector.tensor_scalar(ksl, km[:], -1.47991565,
                                        2.07556761, OP.mult, OP.add)
                nc.vector.scalar_tensor_tensor(
                    ksl, kt1[:], 0.41306651, ksl, op0=OP.mult, op1=OP.add)
                nc.vector.tensor_scalar_max(ksl, ksl, 0.05)
                for _ in range(2):
                    nc.vector.tensor_tensor(kt1[:], ksl, ksl, op=OP.mult)
                    nc.vector.scalar_tensor_tensor(
                        kt1[:], kt1[:], -0.5, km[:], op0=OP.mult, op1=OP.mult)
                    nc.vector.tensor_scalar(kt1[:], kt1[:], 1.0, 1.5,
                                            OP.mult, OP.add)
                    nc.vector.tensor_tensor(ksl, ksl, kt1[:], op=OP.mult)
                nc.vector.tensor_scalar_mul(ksl, ksl, 1.0 / math.sqrt(HALF))


            def make_op_groups(b, tb_list):
                """Per-(tb,cc) out-proj emitters: 3 DoubleRow matmuls + one
                descale copy; the tok-block's store DMA rides the last cc."""
                groups = []
                for tb in tb_list:
                    orow = osb_pool.tile([P, c_dim], bf16, tag="orow",
                                         name="orow")
                    for cc in range(c_dim // 512):
                        def g(tb=tb, cc=cc, orow=orow, b=b):
                            row0 = b * t_dim + tb * P
                            yth_t, ytl_t = ytp[(b, tb // jpc)]
                            jj = tb % jpc
                            op_ps = psum_proj.tile([P, 512], f32, tag="pp",
                                                   name="ops")
                            tsl = slice(jj * P, (jj + 1) * P)
                            csl = slice(cc * 512, (cc + 1) * 512)
                            nc.tensor.matmul(op_ps[:], yth_t[:, 0:2, tsl],
                                             woth_sb[:, 0:2, csl],
                                             start=True, stop=False,
                                             perf_mode=DR)
                            nc.tensor.matmul(op_ps[:], ytl_t[:, 0:2, tsl],
                                             woth_sb[:, 0:2, csl],
                                             start=False, stop=False,
                                             perf_mode=DR)
                            nc.tensor.matmul(op_ps[:], yth_t[:, 0:2, tsl],
                                             wotl_sb[:, 0:2, csl],
                                             start=False, stop=True,
                                             perf_mode=DR)
                            osb = orow[:, cc * 512:(cc + 1) * 512]
                            oc = o["oproj_copy"]
                            if oc == "alt":
                                oc = "act" if (tb + cc) % 2 == 0 else "dve"
                            if oc == "act":
                                nc.scalar.activation(osb, op_ps[:], AF.Copy,
                                                     scale=OUT_DESCALE)
                            else:
                                nc.vector.tensor_scalar_mul(osb, op_ps[:],
                                                            OUT_DESCALE)
                            if cc == c_dim // 512 - 1:
                                out_eng = (nc.sync if o["out_queue"] == "sync"
                                           else nc.scalar)
                                out_eng.dma_start(
                                    out.ap()[row0:row0 + P, :], orow[:])
                        groups.append(g)
                return groups

            def emit_attn(b, cqi, op_b=None, op_blocks=()):
                mark(f"b{b}_attn_c{cqi}")
                op_groups = make_op_groups(op_b if op_b is not None else b,
                                           op_blocks)
                gi = 0
                jmax_ = jpc * cqi + (jpc - 1)
                n_iters = 2 * (jmax_ + 1)
                it = 0
                t_ = batch_tiles(b)
                kt_sb, qt_sb, v_sb, ksc_sb = t_["kt"], t_["qt"], t_["v"], t_["ksc"]
                q0 = cqi * QCH
                jmax = jpc * cqi + (jpc - 1)   # top kk-tile in this chunk
                y0s = []
                yfs = []
                for v in range(2):
                    ys = [psum_y.tile([P, 257], f32, tag="y", name="ys")
                          for _ in range(jpc)]
                    for i in range(jmax + 1):
                        # jj0: first valid j-slot for this row (causal)
                        jj0 = max(0, i - jpc * cqi) if o["narrow_top"] else 0
                        w = QCH - jj0 * P
                        st = psum_st.tile([P, QCH], f32, tag="st",
                                          name="st")[:, :w]
                        nc.tensor.matmul(
                            st[:], kt_sb[:, v, i * P:(i + 1) * P],
                            qt_sb[:, v, q0 + jj0 * P:q0 + QCH],
                            start=True, stop=True)
                        pt = pt_pool.tile([P, QCH], bf16, tag="pt",
                                          name="pt")[:, :w]
                        nc.scalar.activation(
                            pt[:], st[:], AF.Exp,
                            scale=ksc_sb[:, 2 * i + v:2 * i + v + 1])
                        dj = i - jpc * cqi    # diagonal j-slot if >= 0
                        if dj >= 0:
                            nc.vector.tensor_tensor(
                                pt[:, (dj - jj0) * P:(dj - jj0 + 1) * P],
                                pt[:, (dj - jj0) * P:(dj - jj0 + 1) * P],
                                trimask[:], op=OP.mult)
                        for jj in range(jj0, jpc):
                            j = jpc * cqi + jj
                            if i > j:
                                continue
                            nc.tensor.matmul(
                                ys[jj][:],
                                pt[:, (jj - jj0) * P:(jj - jj0 + 1) * P],
                                v_sb[:, i, 0:257],
                                start=(i == 0), stop=(i == j))
                        it += 1
                        while (gi < len(op_groups)
                               and gi + 1 <= len(op_groups) * it // n_iters):
                            op_groups[gi]()
                            gi += 1
                    if v == 0:
                        flush_pending()
                    # epilogue for this view
                    for jj in range(jpc):
                        j = jpc * cqi + jj
                        inv = rms_pool.tile([P, 1], f32, tag="inv")
                        nc.vector.reciprocal(inv[:], ys[jj][:, 256:257])
                        if v == 0:
                            y0 = y0_pool.tile([P, 256], f32, tag="y0")
                            nc.vector.tensor_scalar_mul(
                                y0[:], ys[jj][:, 0:256], inv[:])
                            y0s.append(y0)
                        else:
                            sc2 = rms_pool.tile([P, 1], f32, tag="inv")
                            nc.vector.tensor_tensor(
                                sc2[:], inv[:], lam_sb[:], op=OP.mult)
                            yf = qn_pool.tile([P, 256], bf16, tag="yf")
                            nc.vector.scalar_tensor_tensor(
                                yf[:], ys[jj][:, 0:256], sc2[:], y0s[jj][:],
                                op0=OP.mult, op1=OP.add)
                            yfs.append((j, yf))
                while gi < len(op_groups):
                    op_groups[gi]()
                    gi += 1

                def defer_fn(yfs=yfs, b=b, cqi=cqi):
                    yth_t = yth_pool.tile([P, 2, QCH], f8, tag="yth")
                    ytl_t = ytl_pool.tile([P, 2, QCH], f8, tag="ytl")
                    ytp[(b, cqi)] = (yth_t, ytl_t)
                    for j, yf in yfs:
                        jj = j - jpc * cqi
                        for e in range(2):
                            if o["ytr_pool"] == "y":
                                trp = psum_y.tile([P, 257], bf16, tag="y",
                                                  name="trpy")[:, :P]
                            else:
                                trp = tr_psum.tile([P, tr_shape], bf16,
                                                   tag=tr_tag, name="trp")[:, :P]
                            nc.tensor.transpose(trp, yf[:, e * P:(e + 1) * P],
                                                ident[:])
                            hdst = yth_t[:, e, jj * P:(jj + 1) * P]
                            ldst = ytl_t[:, e, jj * P:(jj + 1) * P]
                            nc.vector.tensor_scalar_mul(hdst, trp, SY)
                            nc.vector.scalar_tensor_tensor(
                                ldst, trp, SY, hdst,
                                op0=OP.mult, op1=OP.subtract)
                pending.append(defer_fn)

            def emit_oproj(b, tb_lo, tb_hi):
                mark(f"b{b}_oproj_tb{tb_lo}")
                groups = make_op_groups(b, range(tb_lo, tb_hi))
                for k, g in enumerate(groups):
                    g()
                    if k == 0:
                        flush_pending()
                flush_pending()

            # ---------------- unit schedule ----------------
            # op-block payloads (owner batch, tok-block list) ride attention
            # units two-plus units after the blocks' own attention chunk; b0's
            # last blocks drain inside b1's early units, b1's in two tail
            # units.
            units = [
                ("proj", 0, 0), ("proj", 0, 1), ("attn", 0, 0, None),
                ("proj", 0, 2), ("attn", 0, 1, None), ("proj", 0, 3),
                ("attn", 0, 2, (0, (0, 1))), ("attn", 0, 3, (0, (2, 3))),
                ("attn", 0, 4, (0, (4, 5))), ("attn", 0, 5, (0, (6, 7))),
                ("attn", 0, 6, (0, (8, 9))), ("attn", 0, 7, (0, (10, 11))),
                ("proj", 1, 0), ("proj", 1, 1),
                ("attn", 1, 0, (0, (12,))), ("proj", 1, 2),
                ("attn", 1, 1, (0, (13,))), ("proj", 1, 3),
                ("attn", 1, 2, (0, (14, 15))), ("attn", 1, 3, (1, (0, 1))),
                ("attn", 1, 4, (1, (2, 3))), ("attn", 1, 5, (1, (4, 5))),
                ("attn", 1, 6, (1, (6, 7))), ("attn", 1, 7, (1, (8, 9))),
                ("oproj", 1, (10, 13)), ("oproj", 1, (13, 16)),
            ]

            for idx, unit in enumerate(units):
                # prefetch x chunks a few units ahead (in order)
                for u2 in units[idx:idx + o["prefetch"]]:
                    if u2[0] == "proj":
                        issue_xc(u2[1], u2[2])
                kind, b = unit[0], unit[1]
                if kind == "proj":
                    emit_proj(b, unit[2])
                elif kind == "attn":
                    payload = unit[3]
                    if payload is None:
                        emit_attn(b, unit[2])
                    else:
                        ob, blocks = payload
                        emit_attn(b, unit[2], op_b=ob, op_blocks=blocks)
                else:
                    emit_oproj(b, unit[2][0], unit[2][1])
                if idx == 1:
                    # out-proj weights: deferred so early x prefetch wins the
                    # DMA device first
                    nc.scalar.dma_start(woth_sb[:], woth_r[:])
                    nc.scalar.dma_start(wotl_sb[:], wotl_r[:])
            flush_pending()
    nc.compile()
    return nc


_NC_CACHE = {}
TRACE = False        # set True (e.g. from test.py) to capture an NTFF profile
LAST_RESULT = None   # BassKernelResults of the most recent run


def _get_nc(c_dim, t_dim, b_dim, **opts):
    key = (c_dim, t_dim, b_dim, tuple(sorted(opts.items())))
    if key not in _NC_CACHE:
        _NC_CACHE[key] = build_nc(c_dim, t_dim, b_dim, **opts)
    return _NC_CACHE[key]


def _fp8_pair(a):
    """hi + residual lo decomposition into e4m3 (values must be in range)."""
    import ml_dtypes
    E4 = ml_dtypes.float8_e4m3
    a = np.clip(a, -224.0, 224.0)
    hi = a.astype(E4)
    lo = (a - hi.astype(np.float32)).astype(E4)
    return hi, lo


def prep_inputs(x, wq, wk, wv, wo, lq1, lk1, lq2, lk2):
    """Host-side prep: per-core input maps (fp8 hi/lo operand pairs)."""
    b_dim, t_dim, c_dim = x.shape
    n_ctiles = c_dim // P

    lam1 = np.exp(np.sum(lq1.astype(np.float64) * lk1.astype(np.float64)))
    lam2 = np.exp(np.sum(lq2.astype(np.float64) * lk2.astype(np.float64)))
    lam_full = np.float32(lam1 - lam2 + LAMBDA_INIT)

    xt = np.ascontiguousarray(
        x.reshape(b_dim * t_dim, c_dim).T).astype(np.float32) * SX
    xh, xl = _fp8_pair(xt)
    lamneg = np.full((P, 1), -lam_full, dtype=np.float32)

    in_maps = []
    for h in range(N_CORES):
        sl = slice(h * HEAD_DIM, (h + 1) * HEAD_DIM)
        wqv = np.concatenate([wq[sl].T, wv[sl].T], axis=1) * SW
        wqvh, wqvl = _fp8_pair(np.ascontiguousarray(wqv))
        # wk pre-swizzled to [128, n_ctiles * 256]
        wk_h = (wk[sl].T * SW).reshape(n_ctiles, P, 256).transpose(1, 0, 2)
        wk_h = np.ascontiguousarray(wk_h).reshape(P, n_ctiles * 256)
        wkh, wkl = _fp8_pair(wk_h)
        wot = np.ascontiguousarray(
            (wo[:, sl] * ((1.0 - LAMBDA_INIT) * SWO)).T)
        woth, wotl = _fp8_pair(wot)
        in_maps.append({
            "xh": xh, "xl": xl, "wqvh": wqvh, "wqvl": wqvl,
            "wkh": wkh, "wkl": wkl, "woth": woth, "wotl": wotl,
            "lamneg": lamneg,
        })
    return in_maps


_FN_CACHE = {}


def _get_callable(nc):
    """Build (once) a reusable jitted shard_map callable for this module —
    mirrors bass2jax.run_bass_via_pjrt's multi-core path, but cached so
    repeat kernel() calls skip retracing."""
    if id(nc) in _FN_CACHE:
        return _FN_CACHE[id(nc)]
    import jax
    from jax.sharding import Mesh, PartitionSpec, NamedSharding
    from jax.experimental.shard_map import shard_map
    import concourse.mybir as mybir
    import concourse.bass2jax as b2j

    b2j.install_neuronx_cc_hook()
    pname = nc.partition_id_tensor.name if nc.partition_id_tensor else None
    in_names, out_names, out_avals, zero_shapes = [], [], [], []
    for alloc in nc.m.functions[0].allocations:
        if not isinstance(alloc, mybir.MemoryLocationSet):
            continue
        name = alloc.memorylocations[0].name
        if alloc.kind == "ExternalInput":
            if name != pname:
                in_names.append(name)
        elif alloc.kind == "ExternalOutput":
            out_names.append(name)
            shape = tuple(alloc.tensor_shape)
            dtype = mybir.dt.np(alloc.dtype)
            out_avals.append(jax.core.ShapedArray(shape, dtype))
            zero_shapes.append((shape, dtype))
    n_params = len(in_names)
    all_in = in_names + out_names
    if pname is not None:
        all_in = all_in + [pname]

    def _body(*args):
        operands = list(args)
        if pname is not None:
            operands.append(b2j.partition_id_tensor())
        return tuple(b2j._bass_exec_p.bind(
            *operands,
            out_avals=tuple(out_avals),
            in_names=tuple(all_in),
            out_names=tuple(out_names),
            lowering_input_output_aliases=(),
            sim_require_finite=True,
            sim_require_nnan=True,
            nc=nc,
        ))

    devices = jax.devices()[:N_CORES]
    mesh = Mesh(np.asarray(devices), ("core",))
    nio = n_params + len(out_names)
    fn = jax.jit(shard_map(_body, mesh=mesh,
                           in_specs=(PartitionSpec("core"),) * nio,
                           out_specs=(PartitionSpec("core"),) * len(out_names),
                           check_rep=False),
                 donate_argnums=tuple(range(n_params, nio)), keep_unused=True)
    sh = NamedSharding(mesh, PartitionSpec("core"))
    entry = (fn, in_names, out_names, zero_shapes, sh)
    _FN_CACHE[id(nc)] = entry
    return entry


def kernel(x, wq, wk, wv, wo, lq1, lk1, lq2, lk2):
    b_dim, t_dim, c_dim = x.shape
    in_maps = prep_inputs(x, wq, wk, wv, wo, lq1, lk1, lq2, lk2)
    nc = _get_nc(c_dim, t_dim, b_dim)

    try:
        import jax
        fn, in_names, out_names, zero_shapes, sh = _get_callable(nc)
        concat_in = [
            np.concatenate([np.asarray(in_maps[c][n]) for c in range(N_CORES)],
                           axis=0) for n in in_names]
        concat_zeros = [np.zeros((N_CORES * s[0], *s[1:]), d)
                        for s, d in zero_shapes]
        dev_in = [jax.device_put(a, sh) for a in concat_in]
        dev_zero = [jax.device_put(a, sh) for a in concat_zeros]
        outs = fn(*dev_in, *dev_zero)
        arr = np.asarray(outs[out_names.index("out")])
        acc = arr.reshape(N_CORES, b_dim * t_dim, c_dim).astype(
            np.float32).sum(axis=0)
    except Exception:
        from concourse.bass_utils import run_bass_kernel_spmd
        res = run_bass_kernel_spmd(nc, in_maps, list(range(N_CORES)),
                                   trace=TRACE)
        global LAST_RESULT
        LAST_RESULT = res
        acc = np.zeros((b_dim * t_dim, c_dim), dtype=np.float32)
        for h in range(N_CORES):
            acc += res.results[h]["out"].astype(np.float32)
    return acc.reshape(b_dim, t_dim, c_dim)


# revision 64
# speedup vs baseline: 1.2983x; 1.0643x over previous
"""Differential attention (B=2, T=2048, C=2048, 8 heads x 256) on 8 trn2 cores.

Sharding: tensor-parallel over the 8 effective heads — core h computes head h's
projections + attention and a partial output projection; host sums bf16
partials in f32.

Per-core pipeline (fp8 DoubleRow matmuls with hi/lo error compensation for the
projections and output projection; bf16 for QK^T and PV):
  xh/xl    [C, B*T]  fp8 e4m3 of x.T*16 (hi) and its residual (lo)
  wqvh/l   [C, 512]  head slice of [wq|wv].T * 1024, fp8 hi/lo
  wkh/l    [128, 16, 256]  head slice of wk.T * 1024 (pre-swizzled), fp8 hi/lo
  woth/l   [256, C]  head slice of wo.T * 0.2 * 4096, fp8 hi/lo
  A @ B is computed as Ah@Bh + Al@Bh + Ah@Bl, each pair of 128-contraction
  tiles packed into one DoubleRow matmul (0.5 cycles/row) -> 0.75x the bf16
  cost with ~bf16 accuracy. Scales are powers of two, folded into the existing
  descale points (rms-norm Rsqrt scale, V/KT psum evacuation, out-proj copy).

Emission is interleaved into units (proj chunk / attention q-chunk / out-proj
block pair) so the in-order PE stream always has ready matmuls; the PE
transposes trailing each unit are deferred one unit ("pending") so their
ACT/DVE producer chains overlap earlier matmul groups.

Attention math: scores computed transposed (S.T[kk,q] = K_tile.T @ Q), exp on
ACT with per-partition kscale, P.T tiles feed PV as lhsT, ones-column on V
gives the softmax denominator. Causal blocks skipped; diagonal masked
multiplicatively post-exp.
"""

import math
from contextlib import ExitStack

import numpy as np

# ---- problem constants (hardcoded per the harness contract) ----
B = 2
T = 2048
C = 2048
N_HEAD = 8
HEAD_DIM = 256
HALF = 128
LAMBDA_INIT = 0.8
RMS_EPS = 1.1920929e-07
N_CORES = 8

P = 128          # partitions
TOK_CHUNK = 512  # projection tok chunk (DMA granularity)

# fp8 scale plan (all powers of two; see module docstring)
SX = 16.0        # x -> x*SX before e4m3
SW = 1024.0      # wq|wv|wk -> *SW
SWO = 4096.0     # wo*0.2 -> *SWO
SY = 8.0         # y -> y*SY at the hi/lo split
QV_DESCALE = 1.0 / (SX * SW)              # 2^-14, applied at V/KT evacuation
RMS_SSQ_SCALE = QV_DESCALE * QV_DESCALE   # 2^-28, folded into rms Rsqrt scale
OUT_DESCALE = 1.0 / (SY * SWO)            # 2^-15, applied at out-proj copy

DEFAULT_OPTS = dict(
    att_chunk=512,       # attention q-chunk width (256 or 512)
    qk_tr="pe",          # "pe" | "dma": Q/K transpose path
    oproj_copy="alt",    # out-proj PSUM->SBUF evacuation: "act"|"dve"|"alt"
    psum=(3, 3, 2),      # banks: (proj, st, y) — must sum to <= 8
    pt_bufs=26,           # P.T tile double-buffer depth
    xc_bufs=3,           # x chunk prefetch depth
    vcopy="dve",         # "act" | "dve": V PSUM->SBUF descale-copy engine
    osb_merge=True,      # one output-store DMA per tok block (vs per c-chunk)
    narrow_top=True,     # compute only the valid half of the top causal row
    tr_pool="st",        # "st" | "pp": PSUM pool used by PE transposes
    tail_split=True,    # stream the final block's stores per c-chunk
    qn_bufs=12,
    y0_mult=2,
    ksq_eng="dve",       # "act" | "dve": engine computing k^2
    ktcopy_eng="dve",    # "act" | "dve": engine evacuating KT psum
    ytr_pool="y",        # "st" | "y": PSUM pool for the y transposes
    out_queue="sync",    # "sync" | "scalar": HWDGE queue for output stores
    prefetch=6,
    warmup=0,
    rampfill=12,
    pv_split=1,   # PV pass A covers jj < split; pass B the rest          # unit lookahead for x chunk DMA issue
)


PHASES = []      # (label, first_instruction_id) marks recorded during build


def build_nc(c_dim, t_dim, b_dim, **opts):
    """Build the per-core Bass module. All shapes in tokens/channels."""
    import concourse.mybir as mybir
    import concourse.tile as tile
    from concourse import bacc
    from concourse.masks import make_identity, make_upper_triangular

    o = dict(DEFAULT_OPTS)
    o.update(opts)
    QCH = o["att_chunk"]
    jpc = QCH // P  # j-blocks per attention chunk

    dt = mybir.dt
    f32 = dt.float32
    bf16 = dt.bfloat16
    f8 = dt.float8e4
    AF = mybir.ActivationFunctionType
    OP = mybir.AluOpType
    DR = mybir.MatmulPerfMode.DoubleRow

    n_ctiles = c_dim // P            # contraction tiles over C
    n_cpairs = n_ctiles // 2         # DoubleRow c-tile pairs
    ntok = b_dim * t_dim             # total token rows
    n_blocks_b = t_dim // P          # 128-tok blocks per batch
    n_qchunks = t_dim // QCH         # attention q chunks per batch
    blocks_per_chunk = TOK_CHUNK // P
    n_chunks = t_dim // TOK_CHUNK
    VP = 272                         # V tile pitch (256 vals + 1 ones + pad)

    nc = bacc.Bacc()

    def mark(label):
        PHASES.append((label, nc.next_id()))

    xh = nc.declare_dram_parameter("xh", [c_dim, ntok], f8, isOutput=False)
    xl = nc.declare_dram_parameter("xl", [c_dim, ntok], f8, isOutput=False)
    wqvh = nc.declare_dram_parameter("wqvh", [c_dim, 512], f8, isOutput=False)
    wqvl = nc.declare_dram_parameter("wqvl", [c_dim, 512], f8, isOutput=False)
    # wk arrives pre-swizzled: [128 partitions, 16 ctiles, 256] row-major so a
    # partition's whole slab is one contiguous 4KB descriptor.
    wkh = nc.declare_dram_parameter("wkh", [P, n_ctiles * 256], f8, isOutput=False)
    wkl = nc.declare_dram_parameter("wkl", [P, n_ctiles * 256], f8, isOutput=False)
    woth = nc.declare_dram_parameter("woth", [HEAD_DIM, c_dim], f8, isOutput=False)
    wotl = nc.declare_dram_parameter("wotl", [HEAD_DIM, c_dim], f8, isOutput=False)
    lamneg = nc.declare_dram_parameter("lamneg", [P, 1], f32, isOutput=False)
    out = nc.declare_dram_parameter("out", [ntok, c_dim], bf16, isOutput=True)

    xh_r = xh.ap().rearrange("(i p) t -> p i t", p=P)      # [128, n_ctiles, ntok]
    xl_r = xl.ap().rearrange("(i p) t -> p i t", p=P)
    wqvh_r = wqvh.ap().rearrange("(i p) n -> p i n", p=P)  # [128, n_ctiles, 512]
    wqvl_r = wqvl.ap().rearrange("(i p) n -> p i n", p=P)
    wkh_r = wkh.ap().rearrange("p (i n) -> p i n", n=256)  # [128, n_ctiles, 256]
    wkl_r = wkl.ap().rearrange("p (i n) -> p i n", n=256)
    woth_r = woth.ap().rearrange("(e p) n -> p e n", p=P)  # [128, 2, c_dim]
    wotl_r = wotl.ap().rearrange("(e p) n -> p e n", p=P)

    with tile.TileContext(nc) as tc:
        with ExitStack() as ctx:
            # ---- persistent SBUF ----
            const_pool = ctx.enter_context(tc.tile_pool(name="const", bufs=1))
            wqvh_sb = const_pool.tile([P, n_ctiles, 512], f8, name="wqvh_sb")
            wqvl_sb = const_pool.tile([P, n_ctiles, 512], f8, name="wqvl_sb")
            wkh_sb = const_pool.tile([P, n_ctiles, 256], f8, name="wkh_sb")
            wkl_sb = const_pool.tile([P, n_ctiles, 256], f8, name="wkl_sb")
            woth_sb = const_pool.tile([P, 2, c_dim], f8, name="woth_sb")
            wotl_sb = const_pool.tile([P, 2, c_dim], f8, name="wotl_sb")
            lam_sb = const_pool.tile([P, 1], f32, name="lam_sb")
            ident = const_pool.tile([P, P], bf16, name="ident")
            trimask = const_pool.tile([P, P], bf16, name="trimask")
            ones_sb = const_pool.tile([P, 1], bf16, name="ones_sb")
            nc.vector.memset(ones_sb[:], 1.0)

            xch_pool = ctx.enter_context(tc.tile_pool(name="xch", bufs=o["xc_bufs"]))
            xcl_pool = ctx.enter_context(tc.tile_pool(name="xcl", bufs=o["xc_bufs"]))

            # ---- ramp: first matmul's operands first; weights on the ACT
            # queue, x chunks + output stores on the SP queue ----
            nc.scalar.dma_start(wkh_sb[:], wkh_r[:])
            xc0h = xch_pool.tile([P, n_ctiles, TOK_CHUNK], f8, tag="xch",
                                 name="xc0h")
            for pc in range(4):     # stream the very first chunk in quarters
                nc.sync.dma_start(xc0h[:, 4 * pc:4 * pc + 4, :],
                                  xh_r[:, 4 * pc:4 * pc + 4, 0:TOK_CHUNK])
            nc.scalar.dma_start(wkl_sb[:], wkl_r[:])
            xc0l = xcl_pool.tile([P, n_ctiles, TOK_CHUNK], f8, tag="xcl",
                                 name="xc0l")
            for pc in range(4):
                nc.sync.dma_start(xc0l[:, 4 * pc:4 * pc + 4, :],
                                  xl_r[:, 4 * pc:4 * pc + 4, 0:TOK_CHUNK])
            nc.scalar.dma_start(wqvh_sb[:], wqvh_r[:])
            nc.scalar.dma_start(wqvl_sb[:], wqvl_r[:])
            nc.scalar.dma_start(lam_sb[:], lamneg.ap())
            make_identity(nc, ident[:])
            # 1.0 where kk <= q (partition <= free), else 0
            make_upper_triangular(nc, trimask[:], val=1.0, diag=True)

            qt_pool = ctx.enter_context(tc.tile_pool(name="qt", bufs=2))
            kt_pool = ctx.enter_context(tc.tile_pool(name="kt", bufs=2))
            ksq_pool = ctx.enter_context(tc.tile_pool(name="ksq", bufs=2))
            kscale_pool = ctx.enter_context(tc.tile_pool(name="kscale", bufs=2))
            v_pool = ctx.enter_context(tc.tile_pool(name="v", bufs=2))
            yth_pool = ctx.enter_context(tc.tile_pool(name="yth", bufs=5))
            ytl_pool = ctx.enter_context(tc.tile_pool(name="ytl", bufs=5))
            pt_pool = ctx.enter_context(tc.tile_pool(name="pt", bufs=o["pt_bufs"]))
            y0_pool = ctx.enter_context(tc.tile_pool(name="y0", bufs=o["y0_mult"] * jpc))
            osb_pool = ctx.enter_context(tc.tile_pool(name="osb", bufs=3))
            qn_pool = ctx.enter_context(tc.tile_pool(name="qn", bufs=o["qn_bufs"]))
            sq_pool = ctx.enter_context(tc.tile_pool(name="sq", bufs=2))
            qcp_pool = ctx.enter_context(tc.tile_pool(name="qcp", bufs=8))
            rms_pool = ctx.enter_context(tc.tile_pool(name="rms", bufs=12))
            nproj, nst, ny = o["psum"]
            psum_proj = ctx.enter_context(
                tc.tile_pool(name="psum_proj", bufs=nproj, space="PSUM"))
            psum_st = ctx.enter_context(
                tc.tile_pool(name="psum_st", bufs=nst, space="PSUM"))
            psum_y = ctx.enter_context(
                tc.tile_pool(name="psum_y", bufs=ny, space="PSUM"))

            tr_psum = psum_st if o["tr_pool"] == "st" else psum_proj
            tr_tag = o["tr_pool"] if o["tr_pool"] == "st" else "pp"
            tr_shape = 256 if o["tr_pool"] == "st" else 512

            # PE p-state warm-up: keep the tensor engine continuously busy
            # with junk transposes while the first DMAs land, so the real
            # matmuls start at full clock (cost model p-state ramp).
            for _ in range(o["warmup"]):
                wtrp = tr_psum.tile([P, tr_shape], bf16, tag=tr_tag,
                                    name="wtrp")[:, :P]
                nc.tensor.transpose(wtrp, ident[:], ident[:])

            def pe_transpose(dst_ap, src_ap):
                trp = tr_psum.tile([P, tr_shape], bf16, tag=tr_tag,
                                   name="trp")[:, :P]
                nc.tensor.transpose(trp, src_ap, ident[:])
                nc.vector.tensor_copy(dst_ap, trp)

            tr_qk = pe_transpose

            # ---------------- per-batch tile state ----------------
            bt = {}

            def batch_tiles(b):
                if b not in bt:
                    bt[b] = dict(
                        qt=qt_pool.tile([P, 2, t_dim], bf16, name=f"qt_b{b}", tag="qt"),
                        kt=kt_pool.tile([P, 2, t_dim], bf16, name=f"kt_b{b}", tag="kt"),
                        v=v_pool.tile([P, n_blocks_b, VP], bf16, name=f"v_b{b}", tag="v"),
                        ksc=kscale_pool.tile([P, 2 * n_blocks_b], f32,
                                             name=f"ksc_b{b}", tag="ksc"),
                    )
                return bt[b]

            pending = []     # deferred emitters: (enqueue_unit_idx, fn)
            cur_unit = [0]

            def flush_pending(min_age=2):
                keep = []
                for enq, fn in pending:
                    if cur_unit[0] - enq >= min_age:
                        fn()
                    else:
                        keep.append((enq, fn))
                pending[:] = keep

            # x chunk DMA management (prefetched ahead of the unit stream)
            xc_tiles = {(0, 0): (xc0h, xc0l)}
            ytp = {}     # (b, attn chunk) -> per-chunk (yth, ytl) fp8 tiles

            def issue_xc(b, ch):
                if (b, ch) in xc_tiles:
                    return
                tok0 = b * t_dim + ch * TOK_CHUNK
                xch = xch_pool.tile([P, n_ctiles, TOK_CHUNK], f8, tag="xch")
                xcl = xcl_pool.tile([P, n_ctiles, TOK_CHUNK], f8, tag="xcl")
                nc.sync.dma_start(xch[:], xh_r[:, :, tok0:tok0 + TOK_CHUNK])
                nc.sync.dma_start(xcl[:], xl_r[:, :, tok0:tok0 + TOK_CHUNK])
                xc_tiles[(b, ch)] = (xch, xcl)

            # ---------------- unit emitters ----------------
            def proj_segments(b, ch):
                """Split one projection chunk into ~2.5us emission segments so
                attention units can interleave them as PE filler."""
                t_ = batch_tiles(b)
                kt_sb, qt_sb, v_sb, ksc_sb = t_["kt"], t_["qt"], t_["v"], t_["ksc"]
                xch, xcl = xc_tiles.pop((b, ch))
                st_ = {"ksqs": [], "qcps": [], "rmsg": None}

                def seg_k(v):
                    if v == 0:
                        mark(f"b{b}_proj_ch{ch}")
                    ktp = psum_proj.tile([P, 512], f32, tag="pp", name="ktp")
                    idx = 0
                    for X, W in ((xch, wkh_sb), (xch, wkl_sb), (xcl, wkh_sb)):
                        for pi in range(n_cpairs):
                            nc.tensor.matmul(
                                ktp[:],
                                W[:, 2 * pi:2 * pi + 2, v * P:(v + 1) * P],
                                X[:, 2 * pi:2 * pi + 2, :],
                                start=(idx == 0),
                                stop=(idx == 3 * n_cpairs - 1),
                                perf_mode=DR)
                            idx += 1
                    ktdst = kt_sb[:, v, ch * TOK_CHUNK:(ch + 1) * TOK_CHUNK]
                    if o["ktcopy_eng"] == "act":
                        nc.scalar.activation(ktdst, ktp[:], AF.Copy,
                                             scale=QV_DESCALE)
                    else:
                        nc.vector.tensor_scalar_mul(ktdst, ktp[:], QV_DESCALE)
                    ksq = ksq_pool.tile([P, TOK_CHUNK], bf16, tag="ksq")
                    if o["ksq_eng"] == "act":
                        nc.scalar.activation(ksq[:], ktdst, AF.Square)
                    else:
                        nc.vector.tensor_tensor(ksq[:], ktdst, ktdst, op=OP.mult)
                    st_["ksqs"].append(ksq)

                def seg_qv(tl):
                    # sq accumulates the whole chunk's ssq into one [P,8] tile;
                    # a descaled q copy (qcp) releases the psum early. Newton
                    # rsqrt, qn muls and PE transposes are deferred.
                    if tl == 0:
                        if b == 0 and ch == 0:
                            # keep PE at p-state while the wqv weights land
                            for _ in range(o["rampfill"]):
                                wtrp = tr_psum.tile([P, tr_shape], bf16,
                                                    tag=tr_tag,
                                                    name="wtrp")[:, :P]
                                nc.tensor.transpose(wtrp, ident[:], ident[:])
                        st_["rmsg"] = rms_pool.tile([P, 8], f32, tag="rms", name="rmsg")
                    rmsg = st_["rmsg"]
                    tb = ch * blocks_per_chunk + tl
                    qv = psum_proj.tile([P, 512], f32, tag="pp", name="qv")
                    idx = 0
                    for X, W in ((xch, wqvh_sb), (xcl, wqvh_sb), (xch, wqvl_sb)):
                        for pi in range(n_cpairs):
                            nc.tensor.matmul(
                                qv[:],
                                X[:, 2 * pi:2 * pi + 2, tl * P:(tl + 1) * P],
                                W[:, 2 * pi:2 * pi + 2, 0:512],
                                start=(idx == 0),
                                stop=(idx == 3 * n_cpairs - 1),
                                perf_mode=DR)
                            idx += 1
                    for j in range(2):
                        sq = sq_pool.tile([P, P], bf16, tag="sq", name="sq")
                        nc.scalar.activation(sq[:], qv[:, j * P:(j + 1) * P],
                                             AF.Square,
                                             accum_out=rmsg[:, 2 * tl + j:
                                                            2 * tl + j + 1])
                    qcp = qcp_pool.tile([P, 256], bf16, tag="qcp")
                    nc.vector.tensor_scalar_mul(qcp[:], qv[:, 0:256],
                                                QV_DESCALE)
                    st_["qcps"].append(qcp)
                    # V (+ ones column for the softmax denominator)
                    if o["vcopy"] == "act":
                        nc.scalar.activation(v_sb[:, tb, 0:256], qv[:, 256:512],
                                             AF.Copy, scale=QV_DESCALE)
                    else:
                        nc.vector.tensor_scalar_mul(v_sb[:, tb, 0:256],
                                                    qv[:, 256:512], QV_DESCALE)
                    nc.vector.memset(v_sb[:, tb, 256:257], 1.0)

                def newton8(dst, m, entry_scale, entry_bias, post=None):
                    """dst = 1/sqrt(m*scale + bias) via seed + 2 Newton iters
                    (DVE-only; ACT Sqrt would force act-table swaps)."""
                    km = rms_pool.tile([P, 8], f32, tag="rms", name="nm")
                    nc.vector.tensor_scalar(km[:], m, entry_scale, entry_bias,
                                            OP.mult, OP.add)
                    t1 = rms_pool.tile([P, 8], f32, tag="rms", name="nt")
                    nc.vector.tensor_tensor(t1[:], km[:], km[:], op=OP.mult)
                    nc.vector.tensor_scalar(dst, km[:], -1.47991565,
                                            2.07556761, OP.mult, OP.add)
                    nc.vector.scalar_tensor_tensor(
                        dst, t1[:], 0.41306651, dst, op0=OP.mult, op1=OP.add)
                    nc.vector.tensor_scalar_max(dst, dst, 0.05)
                    for _ in range(2):
                        nc.vector.tensor_tensor(t1[:], dst, dst, op=OP.mult)
                        nc.vector.scalar_tensor_tensor(
                            t1[:], t1[:], -0.5, km[:], op0=OP.mult, op1=OP.mult)
                        nc.vector.tensor_scalar(t1[:], t1[:], 1.0, 1.5,
                                                OP.mult, OP.add)
                        nc.vector.tensor_tensor(dst, dst, t1[:], op=OP.mult)
                    if post is not None:
                        nc.vector.tensor_scalar_mul(dst, dst, post)

                def seg_tail():
                    # kssq matmuls + kscale (per-block k rms)
                    kssq = psum_proj.tile([P, 512], f32, tag="pp",
                                          name="kssq")[:, :8]
                    for v in range(2):
                        for t in range(blocks_per_chunk):
                            nc.tensor.matmul(
                                kssq[:, 2 * t + v:2 * t + v + 1],
                                st_["ksqs"][v][:, t * P:(t + 1) * P],
                                ones_sb[:], start=True, stop=True)
                    ksl = ksc_sb[:, ch * 2 * blocks_per_chunk:
                                 (ch + 1) * 2 * blocks_per_chunk]
                    newton8(ksl, kssq[:], 1.0 / HALF, RMS_EPS,
                            post=1.0 / math.sqrt(HALF))
                    # q rms: 1/sqrt(ssq * 2^-28 / 128 + eps) (qcp holds true q)
                    yv = rms_pool.tile([P, 8], f32, tag="rms", name="yv")
                    newton8(yv[:], st_["rmsg"][:], RMS_SSQ_SCALE / HALF,
                            RMS_EPS)
                    qcps = st_["qcps"]

                    def defer_fn(ch=ch, qcps=qcps, yv=yv):
                        for tl in range(blocks_per_chunk):
                            tb = ch * blocks_per_chunk + tl
                            for j in range(2):
                                qn = qn_pool.tile([P, P], bf16, tag="qn")
                                nc.vector.tensor_scalar_mul(
                                    qn[:], qcps[tl][:, j * P:(j + 1) * P],
                                    yv[:, 2 * tl + j:2 * tl + j + 1])
                                if o["qk_tr"] == "dma":
                                    nc.scalar.dma_start_transpose(
                                        out=qt_sb[:, j, tb * P:(tb + 1) * P],
                                        in_=qn[:])
                                else:
                                    tr_qk(qt_sb[:, j, tb * P:(tb + 1) * P],
                                          qn[:])
                    pending.append((cur_unit[0], defer_fn))

                return ([lambda: seg_k(0), lambda: seg_k(1)]
                        + [lambda tl=tl: seg_qv(tl)
                           for tl in range(blocks_per_chunk)]
                        + [seg_tail])

            def emit_proj(b, ch):
                segs = proj_segments(b, ch)
                segs[0]()
                flush_pending(min_age=1)
                for fn in segs[1:]:
                    fn()

            def make_op_groups(b, tb_list, stream=False, tail=False):
                """Per-(tb,cc) out-proj emitters: 3 DoubleRow matmuls + one
                descale copy; the tok-block's store DMA rides the last cc
                (or one DMA per cc when streaming the drain)."""
                groups = []
                for tb in tb_list:
                    orow = osb_pool.tile([P, c_dim], bf16, tag="orow",
                                         name="orow")
                    for cc in range(c_dim // 512):
                        def g(tb=tb, cc=cc, orow=orow, b=b,
                              stream=stream, tail=tail):
                            row0 = b * t_dim + tb * P
                            yth_t, ytl_t = ytp[(b, tb // jpc)]
                            jj = tb % jpc
                            op_ps = psum_proj.tile([P, 512], f32, tag="pp",
                                                   name="ops")
                            tsl = slice(jj * P, (jj + 1) * P)
                            csl = slice(cc * 512, (cc + 1) * 512)
                            nc.tensor.matmul(op_ps[:], yth_t[:, 0:2, tsl],
                                             woth_sb[:, 0:2, csl],
                                             start=True, stop=False,
                                             perf_mode=DR)
                            nc.tensor.matmul(op_ps[:], ytl_t[:, 0:2, tsl],
                                             woth_sb[:, 0:2, csl],
                                             start=False, stop=False,
                                             perf_mode=DR)
                            nc.tensor.matmul(op_ps[:], yth_t[:, 0:2, tsl],
                                             wotl_sb[:, 0:2, csl],
                                             start=False, stop=True,
                                             perf_mode=DR)
                            osb = orow[:, cc * 512:(cc + 1) * 512]
                            oc = o["oproj_copy"]
                            if oc == "alt":
                                oc = "act" if (tb + cc) % 2 == 0 else "dve"
                            if oc == "act":
                                nc.scalar.activation(osb, op_ps[:], AF.Copy,
                                                     scale=OUT_DESCALE)
                            else:
                                nc.vector.tensor_scalar_mul(osb, op_ps[:],
                                                            OUT_DESCALE)
                            out_eng = (nc.sync if o["out_queue"] == "sync"
                                       else nc.scalar)
                            if stream and tb == n_blocks_b - 1:
                                out_eng.dma_start(
                                    out.ap()[row0:row0 + P,
                                             cc * 512:(cc + 1) * 512], osb)
                            elif cc == c_dim // 512 - 1 and not (
                                    stream and tb == n_blocks_b - 1):
                                out_eng.dma_start(
                                    out.ap()[row0:row0 + P, :], orow[:])
                        groups.append(g)
                return groups

            def emit_attn(b, cqi, op_list=(), fillers=()):
                mark(f"b{b}_attn_c{cqi}")
                flush_pending()
                op_groups = []
                for ob, blocks in op_list:
                    op_groups.extend(make_op_groups(ob, blocks))
                gi = 0
                fi = 0
                jmax_ = jpc * cqi + (jpc - 1)
                n_iters = 4 * 2 * (jmax_ + 1)   # pass A weighted 2x
                it = 0

                def sprinkle():
                    nonlocal gi, fi
                    while (gi < len(op_groups)
                           and gi + 1 <= len(op_groups) * it // n_iters):
                        op_groups[gi]()
                        gi += 1
                    while (fi < len(fillers)
                           and fi + 1 <= len(fillers) * it // n_iters):
                        fillers[fi]()
                        fi += 1

                t_ = batch_tiles(b)
                kt_sb, qt_sb, v_sb, ksc_sb = t_["kt"], t_["qt"], t_["v"], t_["ksc"]
                q0 = cqi * QCH
                jmax = jmax_
                half = o["pv_split"]
                y0s = {}
                yfs = []
                for v in range(2):
                    pts = []
                    for jj_lo, jj_hi in ((0, half), (half, jpc)):
                        ys = {jj: psum_y.tile([P, 257], f32, tag="y", name="ys")
                              for jj in range(jj_lo, jj_hi)}
                        for i in range(jmax + 1):
                            if jj_lo == 0:
                                # pass A: compute scores + exp as we go
                                jj0 = (max(0, i - jpc * cqi)
                                       if o["narrow_top"] else 0)
                                w = QCH - jj0 * P
                                st = psum_st.tile([P, QCH], f32, tag="st",
                                                  name="st")[:, :w]
                                nc.tensor.matmul(
                                    st[:], kt_sb[:, v, i * P:(i + 1) * P],
                                    qt_sb[:, v, q0 + jj0 * P:q0 + QCH],
                                    start=True, stop=True)
                                pt = pt_pool.tile([P, QCH], bf16, tag="pt",
                                                  name="pt")[:, :w]
                                nc.scalar.activation(
                                    pt[:], st[:], AF.Exp,
                                    scale=ksc_sb[:, 2 * i + v:2 * i + v + 1])
                                dj = i - jpc * cqi
                                if dj >= 0:
                                    nc.vector.tensor_tensor(
                                        pt[:, (dj - jj0) * P:(dj - jj0 + 1) * P],
                                        pt[:, (dj - jj0) * P:(dj - jj0 + 1) * P],
                                        trimask[:], op=OP.mult)
                                pts.append((pt, jj0))
                            else:
                                pt, jj0 = pts[i]
                            for jj in range(max(jj_lo, jj0), jj_hi):
                                j = jpc * cqi + jj
                                if i > j:
                                    continue
                                nc.tensor.matmul(
                                    ys[jj][:],
                                    pt[:, (jj - jj0) * P:(jj - jj0 + 1) * P],
                                    v_sb[:, i, 0:257],
                                    start=(i == 0), stop=(i == j))
                            it += 1 if jj_lo != 0 else 3
                            sprinkle()
                        if jj_lo == 0:
                            flush_pending(min_age=1)
                        # epilogue for this pass/view
                        for jj in range(jj_lo, jj_hi):
                            j = jpc * cqi + jj
                            inv = rms_pool.tile([P, 1], f32, tag="inv")
                            nc.vector.reciprocal(inv[:], ys[jj][:, 256:257])
                            if v == 0:
                                y0 = y0_pool.tile([P, 256], f32, tag="y0")
                                nc.vector.tensor_scalar_mul(
                                    y0[:], ys[jj][:, 0:256], inv[:])
                                y0s[jj] = y0
                            else:
                                sc2 = rms_pool.tile([P, 1], f32, tag="inv")
                                nc.vector.tensor_tensor(
                                    sc2[:], inv[:], lam_sb[:], op=OP.mult)
                                yf = qn_pool.tile([P, 256], bf16, tag="yf")
                                nc.vector.scalar_tensor_tensor(
                                    yf[:], ys[jj][:, 0:256], sc2[:],
                                    y0s[jj][:], op0=OP.mult, op1=OP.add)
                                yfs.append((j, yf))
                while gi < len(op_groups):
                    op_groups[gi]()
                    gi += 1
                while fi < len(fillers):
                    fillers[fi]()
                    fi += 1

                def defer_fn(yfs=yfs, b=b, cqi=cqi):
                    yth_t = yth_pool.tile([P, 2, QCH], f8, tag="yth")
                    ytl_t = ytl_pool.tile([P, 2, QCH], f8, tag="ytl")
                    ytp[(b, cqi)] = (yth_t, ytl_t)
                    for j, yf in yfs:
                        jj = j - jpc * cqi
                        for e in range(2):
                            if o["ytr_pool"] == "y":
                                trp = psum_y.tile([P, 257], bf16, tag="y",
                                                  name="trpy")[:, :P]
                            else:
                                trp = tr_psum.tile([P, tr_shape], bf16,
                                                   tag=tr_tag, name="trp")[:, :P]
                            nc.tensor.transpose(trp, yf[:, e * P:(e + 1) * P],
                                                ident[:])
                            hdst = yth_t[:, e, jj * P:(jj + 1) * P]
                            ldst = ytl_t[:, e, jj * P:(jj + 1) * P]
                            nc.vector.tensor_scalar_mul(hdst, trp, SY)
                            nc.vector.scalar_tensor_tensor(
                                ldst, trp, SY, hdst,
                                op0=OP.mult, op1=OP.subtract)
                pending.append((cur_unit[0], defer_fn))

            def emit_oproj(b, tb_lo, tb_hi):
                mark(f"b{b}_oproj_tb{tb_lo}")
                flush_pending(min_age=1)
                groups = make_op_groups(b, range(tb_lo, tb_hi),
                                        stream=o["tail_split"], tail=True)
                for g in groups:
                    g()

            # ---------------- unit schedule ----------------
            # unit schedule: (kind, b, arg, op_payload, fill) — op-block
            # payloads (owner batch, tok-blocks) ride attention units two-plus
            # units after the blocks' own attention chunk.
            units = [
                ("proj", 0, 0), ("proj", 0, 1),
                ("attn", 0, 0, (), None),
                ("proj", 0, 2),
                ("attn", 0, 1, (), None),
                ("proj", 0, 3),
                ("attn", 0, 2, ((0, (0, 1, 2, 3)),), None),
                ("attn", 0, 3, ((0, (4, 5, 6, 7)),), None),
                ("proj", 1, 0), ("proj", 1, 1),
                ("attn", 1, 0, ((0, (8, 9)),), None),
                ("proj", 1, 2),
                ("attn", 1, 1, ((0, (10, 11, 12, 13, 14, 15)),), None),
                ("proj", 1, 3),
                ("attn", 1, 2, ((1, (0, 1, 2, 3)),), None),
                ("attn", 1, 3, ((1, (4, 5, 6, 7)),), None),
                ("oproj", 1, (8, 12)), ("oproj", 1, (12, 16)),
            ]

            for idx, unit in enumerate(units):
                cur_unit[0] = idx
                # prefetch x chunks a few units ahead (in order)
                for u2 in units[idx:idx + o["prefetch"]]:
                    if u2[0] == "proj":
                        issue_xc(u2[1], u2[2])
                    elif u2[0] == "attn" and u2[4] is not None:
                        issue_xc(u2[4][0], u2[4][1])
                kind, b = unit[0], unit[1]
                if kind == "proj":
                    emit_proj(b, unit[2])
                elif kind == "attn":
                    payload, fill = unit[3], unit[4]
                    fillers = (proj_segments(fill[0], fill[1])
                               if fill is not None else ())
                    emit_attn(b, unit[2], op_list=payload, fillers=fillers)
                else:
                    emit_oproj(b, unit[2][0], unit[2][1])
                if idx == 1:
                    # out-proj weights: deferred so early x prefetch wins the
                    # DMA device first
                    nc.scalar.dma_start(woth_sb[:], woth_r[:])
                    nc.scalar.dma_start(wotl_sb[:], wotl_r[:])
            cur_unit[0] += 10
            flush_pending(min_age=0)
    nc.compile()
    return nc


_NC_CACHE = {}
TRACE = False        # set True (e.g. from test.py) to capture an NTFF profile
LAST_RESULT = None   # BassKernelResults of the most recent run


def _get_nc(c_dim, t_dim, b_dim, **opts):
    key = (c_dim, t_dim, b_dim, tuple(sorted(opts.items())))
    if key not in _NC_CACHE:
        _NC_CACHE[key] = build_nc(c_dim, t_dim, b_dim, **opts)
    return _NC_CACHE[key]


def _fp8_pair(a):
    """hi + residual lo decomposition into e4m3 (values must be in range)."""
    import ml_dtypes
    E4 = ml_dtypes.float8_e4m3
    a = np.clip(a, -224.0, 224.0)
    hi = a.astype(E4)
    lo = (a - hi.astype(np.float32)).astype(E4)
    return hi, lo


def prep_inputs(x, wq, wk, wv, wo, lq1, lk1, lq2, lk2):
    """Host-side prep: per-core input maps (fp8 hi/lo operand pairs)."""
    b_dim, t_dim, c_dim = x.shape
    n_ctiles = c_dim // P

    lam1 = np.exp(np.sum(lq1.astype(np.float64) * lk1.astype(np.float64)))
    lam2 = np.exp(np.sum(lq2.astype(np.float64) * lk2.astype(np.float64)))
    lam_full = np.float32(lam1 - lam2 + LAMBDA_INIT)

    xt = np.ascontiguousarray(
        x.reshape(b_dim * t_dim, c_dim).T).astype(np.float32) * SX
    xh, xl = _fp8_pair(xt)
    lamneg = np.full((P, 1), -lam_full, dtype=np.float32)

    in_maps = []
    for h in range(N_CORES):
        sl = slice(h * HEAD_DIM, (h + 1) * HEAD_DIM)
        wqv = np.concatenate([wq[sl].T, wv[sl].T], axis=1) * SW
        wqvh, wqvl = _fp8_pair(np.ascontiguousarray(wqv))
        # wk pre-swizzled to [128, n_ctiles * 256]
        wk_h = (wk[sl].T * SW).reshape(n_ctiles, P, 256).transpose(1, 0, 2)
        wk_h = np.ascontiguousarray(wk_h).reshape(P, n_ctiles * 256)
        wkh, wkl = _fp8_pair(wk_h)
        wot = np.ascontiguousarray(
            (wo[:, sl] * ((1.0 - LAMBDA_INIT) * SWO)).T)
        woth, wotl = _fp8_pair(wot)
        in_maps.append({
            "xh": xh, "xl": xl, "wqvh": wqvh, "wqvl": wqvl,
            "wkh": wkh, "wkl": wkl, "woth": woth, "wotl": wotl,
            "lamneg": lamneg,
        })
    return in_maps


_FN_CACHE = {}


def _get_callable(nc):
    """Build (once) a reusable jitted shard_map callable for this module —
    mirrors bass2jax.run_bass_via_pjrt's multi-core path, but cached so
    repeat kernel() calls skip retracing."""
    if id(nc) in _FN_CACHE:
        return _FN_CACHE[id(nc)]
    import jax
    from jax.sharding import Mesh, PartitionSpec, NamedSharding
    from jax.experimental.shard_map import shard_map
    import concourse.mybir as mybir
    import concourse.bass2jax as b2j

    b2j.install_neuronx_cc_hook()
    pname = nc.partition_id_tensor.name if nc.partition_id_tensor else None
    in_names, out_names, out_avals, zero_shapes = [], [], [], []
    for alloc in nc.m.functions[0].allocations:
        if not isinstance(alloc, mybir.MemoryLocationSet):
            continue
        name = alloc.memorylocations[0].name
        if alloc.kind == "ExternalInput":
            if name != pname:
                in_names.append(name)
        elif alloc.kind == "ExternalOutput":
            out_names.append(name)
            shape = tuple(alloc.tensor_shape)
            dtype = mybir.dt.np(alloc.dtype)
            out_avals.append(jax.core.ShapedArray(shape, dtype))
            zero_shapes.append((shape, dtype))
    n_params = len(in_names)
    all_in = in_names + out_names
    if pname is not None:
        all_in = all_in + [pname]

    def _body(*args):
        operands = list(args)
        if pname is not None:
            operands.append(b2j.partition_id_tensor())
        return tuple(b2j._bass_exec_p.bind(
            *operands,
            out_avals=tuple(out_avals),
            in_names=tuple(all_in),
            out_names=tuple(out_names),
            lowering_input_output_aliases=(),
            sim_require_finite=True,
            sim_require_nnan=True,
            nc=nc,
        ))

    devices = jax.devices()[:N_CORES]
    mesh = Mesh(np.asarray(devices), ("core",))
    nio = n_params + len(out_names)
    fn = jax.jit(shard_map(_body, mesh=mesh,
                           in_specs=(PartitionSpec("core"),) * nio,
                           out_specs=(PartitionSpec("core"),) * len(out_names),
                           check_rep=False),
                 donate_argnums=tuple(range(n_params, nio)), keep_unused=True)
    sh = NamedSharding(mesh, PartitionSpec("core"))
    entry = (fn, in_names, out_names, zero_shapes, sh)
    _FN_CACHE[id(nc)] = entry
    return entry


def kernel(x, wq, wk, wv, wo, lq1, lk1, lq2, lk2):
    b_dim, t_dim, c_dim = x.shape
    in_maps = prep_inputs(x, wq, wk, wv, wo, lq1, lk1, lq2, lk2)
    nc = _get_nc(c_dim, t_dim, b_dim)

    try:
        import jax
        fn, in_names, out_names, zero_shapes, sh = _get_callable(nc)
        concat_in = [
            np.concatenate([np.asarray(in_maps[c][n]) for c in range(N_CORES)],
                           axis=0) for n in in_names]
        concat_zeros = [np.zeros((N_CORES * s[0], *s[1:]), d)
                        for s, d in zero_shapes]
        dev_in = [jax.device_put(a, sh) for a in concat_in]
        dev_zero = [jax.device_put(a, sh) for a in concat_zeros]
        outs = fn(*dev_in, *dev_zero)
        arr = np.asarray(outs[out_names.index("out")])
        acc = arr.reshape(N_CORES, b_dim * t_dim, c_dim).astype(
            np.float32).sum(axis=0)
    except Exception:
        from concourse.bass_utils import run_bass_kernel_spmd
        res = run_bass_kernel_spmd(nc, in_maps, list(range(N_CORES)),
                                   trace=TRACE)
        global LAST_RESULT
        LAST_RESULT = res
        acc = np.zeros((b_dim * t_dim, c_dim), dtype=np.float32)
        for h in range(N_CORES):
            acc += res.results[h]["out"].astype(np.float32)
    return acc.reshape(b_dim, t_dim, c_dim)
